# revision 1
# baseline (speedup 1.0000x reference)
"""DIN attention layer kernel for Trainium2 (8 NeuronCores, data-parallel over batch).

Reference math:
  x  = concat([q, ub, q-ub, q*ub], -1)             # [B,T,144]
  h1 = sigmoid(x @ W1 + b1)                        # [B,T,80]
  h2 = sigmoid(h1 @ W2 + b2)                       # [B,T,40]
  s  = h2 @ W3 + b3                                # [B,T,1]
  w  = softmax(s.T * mask)                         # [B,1,T]  (multiplicative mask)
  out = w @ ub                                     # [B,1,36]

Host-side algebraic folds:
  1) x @ W1 = ub @ (Wb-Wc) + (q*ub) @ Wd + q @ (Wa+Wc); q is per-batch, so fold
     into per-batch weights Waug_b = [(Wb-Wc) + diag(q_b) Wd ; q_b(Wa+Wc)+b1]
     ([37,80]) and augment ub with a ones column -> single K=37 matmul.
  2) sigmoid(x) = 0.5 + 0.5*tanh(x/2); tanh and exp share one ACT table set
     (sigmoid does not), so the device computes t = tanh(pre/2) and the
     0.5/0.5 affine is folded into the next layer's weights/biases.
"""

from contextlib import ExitStack

import numpy as np

import concourse.bass as bass
import concourse.bacc as bacc
import concourse.tile as tile
from concourse import mybir
from concourse.bass_utils import run_bass_kernel_spmd

B, T, E = 4096, 200, 36
N_CORES = 8
F32 = mybir.dt.float32
AF = mybir.ActivationFunctionType


def _segments(start, end, step=128):
    segs = []
    s = start
    while s < end:
        e = min(end, (s // step + 1) * step)
        segs.append((s, e))
        s = e
    return segs


def build_module(bc=512, pb=64, stage=99):
    """bc = batches per core, pb = batches per phase (pb % 32 == 0 keeps
    phases aligned to 128-row tiles, 200-row batches, and 6400-row halves)."""
    assert bc % pb == 0 and pb % 32 == 0 and pb <= 64
    ph_n = bc // pb
    rp = 200 * pb                # rows per phase
    nt = rp // 128               # 128-row tiles per phase
    npair = nt // 2              # transpose pairs per phase
    nhalf = pb // 32             # 32-batch half-phases per phase
    hpair = npair // nhalf       # pairs per half-phase (25)
    sm = pb                      # softmax tile partitions

    nc = bacc.Bacc(
        "TRN2", target_bir_lowering=False, debug=False,
        enable_asserts=False, num_devices=N_CORES,
    )

    ubaug_d = nc.dram_tensor("ubaug", [bc * 200, 37], F32, kind="ExternalInput").ap()
    waug_d = nc.dram_tensor("waug", [bc, 37, 80], F32, kind="ExternalInput").ap()
    lens_d = nc.dram_tensor("lens", [bc, 1], F32, kind="ExternalInput").ap()
    w2_d = nc.dram_tensor("w2", [80, 64], F32, kind="ExternalInput").ap()
    w3d0_d = nc.dram_tensor("w3d0", [104, 32], F32, kind="ExternalInput").ap()
    w3d1_d = nc.dram_tensor("w3d1", [104, 32], F32, kind="ExternalInput").ap()
    b2c_d = nc.dram_tensor("b2c", [128, 1], F32, kind="ExternalInput").ap()
    b3c_d = nc.dram_tensor("b3c", [128, 1], F32, kind="ExternalInput").ap()
    out_d = nc.dram_tensor("out", [bc, 36], F32, kind="ExternalOutput").ap()
    sc_dram = nc.dram_tensor("sc_scratch", [bc * 200], F32, kind="Internal").ap()
    w_dram = nc.dram_tensor("w_scratch", [bc * 200], F32, kind="Internal").ap()

    ident_d = nc.inline_tensor(np.eye(128, dtype=np.float32), name="ident").ap()
    iota_d = nc.inline_tensor(
        np.broadcast_to(np.arange(200, dtype=np.float32), (64, 200)).copy(),
        name="iotat").ap()
    fmA = np.zeros((128, nt), dtype=np.float32)
    fmB = np.zeros((128, nt), dtype=np.float32)
    for k in range(nt):
        b_lo = (128 * k) // 200
        for p in range(128):
            if (128 * k + p) // 200 == b_lo:
                fmA[p, k] = 1.0
            else:
                fmB[p, k] = 1.0
    fmA_d = nc.inline_tensor(fmA, name="fmA").ap()
    fmB_d = nc.inline_tensor(fmB, name="fmB").ap()

    with tile.TileContext(nc) as tc, ExitStack() as es:
        cpool = es.enter_context(tc.tile_pool(name="consts", bufs=1))
        xaugp = es.enter_context(tc.tile_pool(name="xaug", bufs=2))
        h1p = es.enter_context(tc.tile_pool(name="h1p", bufs=4))
        h2p = es.enter_context(tc.tile_pool(name="h2p", bufs=4))
        scbp = es.enter_context(tc.tile_pool(name="scbp", bufs=3))
        smp = es.enter_context(tc.tile_pool(name="smp", bufs=2))
        wcp = es.enter_context(tc.tile_pool(name="wcp", bufs=2))
        outp = es.enter_context(tc.tile_pool(name="outp", bufs=2))
        tpp = es.enter_context(tc.tile_pool(name="tpp", bufs=2, space="PSUM"))
        m1p = es.enter_context(tc.tile_pool(name="m1p", bufs=2, space="PSUM"))
        m23p = es.enter_context(tc.tile_pool(name="m23p", bufs=2, space="PSUM"))

        ident = cpool.tile([128, 128], F32)
        nc.sync.dma_start(out=ident, in_=ident_d)
        iota_t = cpool.tile([64, 200], F32)
        nc.sync.dma_start(out=iota_t, in_=iota_d)
        w2_t = cpool.tile([80, 64], F32)
        nc.sync.dma_start(out=w2_t, in_=w2_d)
        w3_0t = cpool.tile([104, 32], F32)
        nc.sync.dma_start(out=w3_0t, in_=w3d0_d)
        w3_1t = cpool.tile([104, 32], F32)
        nc.sync.dma_start(out=w3_1t, in_=w3d1_d)
        b2_t = cpool.tile([128, 1], F32)
        nc.sync.dma_start(out=b2_t, in_=b2c_d)
        b3_t = cpool.tile([128, 1], F32)
        nc.sync.dma_start(out=b3_t, in_=b3c_d)
        fmA_t = cpool.tile([128, nt], F32)
        nc.sync.dma_start(out=fmA_t, in_=fmA_d)
        fmB_t = cpool.tile([128, nt], F32)
        nc.sync.dma_start(out=fmB_t, in_=fmB_d)
        # zero-padded, manually double-buffered data/weight tensors
        waug0 = cpool.tile([101, 2, 32, 80], F32)
        nc.vector.memset(waug0, 0.0)
        waug1 = cpool.tile([101, 2, 32, 80], F32)
        nc.vector.memset(waug1, 0.0)
        nat2 = cpool.tile([128, 2, npair, 101], F32)
        nc.vector.memset(nat2, 0.0)

        for ph in range(ph_n):
            slot = ph % 2
            # ---- load nat tiles: [128, slot, pair, {0:37 | 64:101}] ----
            for d in range((npair + 9) // 10):
                p0 = 10 * d
                pn = min(10, npair - p0)
                for blk in range(2):
                    nsrc = bass.AP(
                        tensor=ubaug_d.tensor,
                        offset=ubaug_d.offset
                        + 37 * (rp * ph + 256 * p0 + 128 * blk),
                        ap=[[37, 128], [37 * 256, pn], [1, 37]],
                    )
                    nc.sync.dma_start(
                        out=nat2[:, slot, p0:p0 + pn, 64 * blk:64 * blk + 37],
                        in_=nsrc)

            lencol = smp.tile([sm, 1], F32, tag="lencol")
            nc.sync.dma_start(
                out=lencol,
                in_=bass.AP(tensor=lens_d.tensor,
                            offset=lens_d.offset + pb * ph,
                            ap=[[1, sm], [1, 1]]))

            sc_ps = None
            for hp2 in range(nhalf):
                wslot = (ph * nhalf + hp2) % 2
                wsrc = bass.AP(
                    tensor=waug_d.tensor,
                    offset=waug_d.offset + 37 * 80 * (pb * ph + 32 * hp2),
                    ap=[[80, 37], [37 * 80, 32], [1, 80]],
                )
                nc.sync.dma_start(out=waug0[0:37, wslot, :, :], in_=wsrc)
                wsrc2 = bass.AP(
                    tensor=waug_d.tensor,
                    offset=waug_d.offset + 37 * 80 * (pb * ph + 32 * hp2),
                    ap=[[80, 37], [37 * 80, 32], [1, 80]],
                )
                nc.sync.dma_start(out=waug1[64:101, wslot, :, :], in_=wsrc2)

                # ---- transpose this half-phase into xaug [101, hpair*128] ----
                xaug = xaugp.tile([101, hpair * 128], F32, tag="xaug")
                ngrp = (hpair + 3) // 4
                for g in range(ngrp):
                    tp_ps = tpp.tile([101, 512], F32, tag="tps")
                    pg0 = 4 * g
                    pgn = min(4, hpair - pg0)
                    for j in range(pgn):
                        nc.tensor.transpose(
                            tp_ps[0:101, 128 * j:128 * (j + 1)],
                            nat2[:, slot, hpair * hp2 + pg0 + j, :], ident)
                    nc.vector.tensor_copy(
                        out=xaug[0:101, 512 * g:512 * g + 128 * pgn],
                        in_=tp_ps[0:101, 0:128 * pgn])

                if stage <= 1:
                    dbg = h1p.tile([101, 512], F32, tag="dbg1")
                    nc.vector.tensor_copy(out=dbg, in_=xaug[:, 0:512])
                    nc.sync.dma_start(out=out_d[0:32, :], in_=dbg[0:32, 0:36])
                    continue

                # ---- MLP: 4 batches per mm1 psum tile ----
                for q4 in range(8):
                    m1_ps = m1p.tile([80, 1024], F32, tag="m1")
                    for j in range(4):
                        bl32 = 4 * q4 + j          # batch within half-phase
                        bl = 32 * hp2 + bl32       # batch within phase
                        colbase = 512 * (j // 2) + 200 * (j % 2)
                        for (rs, re) in _segments(200 * bl32, 200 * bl32 + 200):
                            k = rs // 128
                            i0 = rs % 128
                            c0 = 128 * (k // 2) + i0
                            wt_sel = waug0 if k % 2 == 0 else waug1
                            nc.tensor.matmul(
                                m1_ps[0:80, colbase + rs - 200 * bl32:
                                      colbase + re - 200 * bl32],
                                wt_sel[0:101, wslot, bl32, :],
                                xaug[0:101, c0:c0 + (re - rs)],
                                start=True, stop=True)
                    if stage == 15:
                        if q4 == 0:
                            dbg2 = h1p.tile([80, 400], F32, tag="dbg2")
                            nc.vector.tensor_copy(out=dbg2,
                                                  in_=m1_ps[0:80, 0:400])
                            nc.sync.dma_start(out=out_d[0:32, :],
                                              in_=dbg2[0:32, 0:36])
                        continue
                    h1_t = h1p.tile([80, 2, 400], F32, tag="h1")
                    nc.scalar.activation(
                        out=h1_t,
                        in_=m1_ps.rearrange("p (u c) -> p u c", u=2)[:, :, 0:400],
                        func=AF.Tanh, scale=0.5)
                    if stage <= 2:
                        if q4 == 0:
                            nc.sync.dma_start(out=out_d[0:32, :],
                                              in_=h1_t[0:32, 0, 0:36])
                        continue
                    m2_ps = m23p.tile([128, 512], F32, tag="m23")
                    for j in range(4):
                        ps = 64 * (j % 2)
                        ch = 200 * (j // 2)
                        nc.tensor.matmul(
                            m2_ps[ps:ps + 64, ch:ch + 200], w2_t,
                            h1_t[:, j // 2, (j % 2) * 200:(j % 2) * 200 + 200],
                            start=True, stop=True)
                    h2_t = h2p.tile([128, 400], F32, tag="h2")
                    nc.scalar.activation(
                        out=h2_t, in_=m2_ps[0:128, 0:400],
                        func=AF.Tanh, bias=b2_t, scale=0.5)
                    if stage <= 3:
                        if q4 == 0:
                            nc.sync.dma_start(out=out_d[0:32, :],
                                              in_=h2_t[0:32, 0:36])
                        continue
                    if q4 % 2 == 0:
                        sc_ps = m23p.tile([128, 400], F32, tag="m23")
                    for j in range(4):
                        bl8 = (4 * q4 + j) % 8
                        pslot = 32 * (bl8 // 2)
                        half = 200 * (bl8 % 2)
                        ps = 64 * (j % 2)
                        ch = 200 * (j // 2)
                        w3_sel = w3_0t if ps == 0 else w3_1t
                        nc.tensor.matmul(
                            sc_ps[pslot:pslot + 32, half:half + 200],
                            w3_sel, h2_t[0:104, ch:ch + 200],
                            start=True, stop=True, tile_position=(0, pslot))
                    if q4 % 2 == 1:
                        scb = scbp.tile([97, 400], F32, tag="scb")
                        nc.vector.tensor_copy(out=scb, in_=sc_ps[0:97, :])
                        g8 = (32 * hp2) // 8 + q4 // 2
                        nc.sync.dma_start(
                            out=bass.AP(
                                tensor=sc_dram.tensor,
                                offset=sc_dram.offset
                                + 200 * (pb * ph + 8 * g8),
                                ap=[[400, 4], [200, 2], [1, 200]]),
                            in_=bass.AP(
                                tensor=scb.tensor, offset=scb.offset,
                                ap=[[400 * 32, 4], [200, 2], [1, 200]]))

            if stage <= 4:
                continue

            # ---- masked softmax over t, batches on partitions ----
            sc_t = smp.tile([sm, 200], F32, tag="sc")
            nc.sync.dma_start(
                out=sc_t,
                in_=bass.AP(tensor=sc_dram.tensor,
                            offset=sc_dram.offset + 200 * pb * ph,
                            ap=[[200, sm], [1, 200]]))
            mask_t = smp.tile([sm, 200], F32, tag="mask")
            nc.vector.tensor_scalar(
                out=mask_t, in0=iota_t[0:sm, :], scalar1=lencol, scalar2=None,
                op0=mybir.AluOpType.is_lt)
            sb3 = smp.tile([sm, 200], F32, tag="sb3")
            nc.vector.tensor_scalar_add(sb3, sc_t, b3_t[0:sm, :])
            masked = smp.tile([sm, 200], F32, tag="masked")
            nc.vector.tensor_mul(masked, sb3, mask_t)
            negmax = smp.tile([sm, 1], F32, tag="negmax")
            nc.vector.tensor_reduce(
                out=negmax, in_=masked, axis=mybir.AxisListType.X,
                op=mybir.AluOpType.max, negate=True)
            ew = smp.tile([sm, 200], F32, tag="ew")
            sumexp = smp.tile([sm, 1], F32, tag="sumexp")
            nc.scalar.activation(
                out=ew, in_=masked, func=AF.Exp,
                bias=negmax, accum_out=sumexp)
            rz = smp.tile([sm, 1], F32, tag="rz")
            nc.vector.reciprocal(rz, sumexp)
            w_t = smp.tile([sm, 200], F32, tag="wt")
            nc.vector.tensor_scalar_mul(w_t, ew, rz)
            nc.sync.dma_start(
                out=bass.AP(tensor=w_dram.tensor,
                            offset=w_dram.offset + rp * ph,
                            ap=[[200, sm], [1, 200]]),
                in_=w_t)

            if stage <= 5:
                continue

            # ---- weighted sum of ub rows (softmax @ ub) ----
            wcols = wcp.tile([128, nt], F32, tag="wcols")
            nc.sync.dma_start(
                out=wcols,
                in_=bass.AP(tensor=w_dram.tensor,
                            offset=w_dram.offset + rp * ph,
                            ap=[[1, 128], [128, nt]]))
            wA = wcp.tile([128, nt], F32, tag="wA")
            nc.vector.tensor_mul(wA, wcols, fmA_t)
            wB = wcp.tile([128, nt], F32, tag="wB")
            nc.vector.tensor_mul(wB, wcols, fmB_t)

            if stage <= 6:
                continue

            n_mv = (pb + 51) // 52
            mv_tiles = []
            for _i in range(n_mv):
                mv_t = m23p.tile([97, 481], F32, tag="m23", name=f"mv{_i}")
                nc.vector.memset(mv_t[0:97, 0:481], 0.0)
                mv_tiles.append(mv_t)

            def emit_mv(bb, w_tile, k, rhs):
                t0 = (200 * bb) // 128
                t1 = (200 * bb + 199) // 128
                mv = mv_tiles[bb // 52]
                bi = bb % 52
                pslot = 32 * (bi % 4)
                colr = 37 * (bi // 4)
                nc.tensor.matmul(
                    mv[pslot:pslot + 1, colr:colr + 37],
                    w_tile[:, k:k + 1], rhs,
                    start=(k == t0), stop=(k == t1),
                    tile_position=(0, pslot))

            for k in range(nt):
                b_lo = (128 * k) // 200
                b_hi = (128 * k + 127) // 200
                rhs = nat2[:, slot, k // 2, 64 * (k % 2):64 * (k % 2) + 37]
                if b_lo == b_hi:
                    emit_mv(b_lo, wcols, k, rhs)
                else:
                    emit_mv(b_lo, wA, k, rhs)
                    emit_mv(b_hi, wB, k, rhs)

            for i in range(n_mv):
                nb = min(52, pb - 52 * i)
                ncolr = (nb + 3) // 4
                o_t = outp.tile([97, 481], F32, tag="out")
                nc.vector.tensor_copy(
                    out=o_t[0:97, 0:37 * ncolr],
                    in_=mv_tiles[i][0:97, 0:37 * ncolr])
                nc.sync.dma_start(
                    out=bass.AP(
                        tensor=out_d.tensor,
                        offset=out_d.offset + 36 * (pb * ph + 52 * i),
                        ap=[[36, min(4, nb)], [144, ncolr], [1, 36]]),
                    in_=bass.AP(
                        tensor=o_t.tensor, offset=o_t.offset,
                        ap=[[481 * 32, min(4, nb)], [37, ncolr], [1, 36]]))

    nc.compile()
    return nc


def host_prep(query_ad, user_behavior, user_behavior_length,
              W1, b1, W2, b2, W3, b3, bc):
    q = np.asarray(query_ad, dtype=np.float32)
    ub = np.asarray(user_behavior, dtype=np.float32)
    lens = np.asarray(user_behavior_length)
    W1 = np.asarray(W1, dtype=np.float32)
    b1 = np.asarray(b1, dtype=np.float32)
    W2 = np.asarray(W2, dtype=np.float32)
    b2 = np.asarray(b2, dtype=np.float32)
    W3 = np.asarray(W3, dtype=np.float32)
    b3 = np.asarray(b3, dtype=np.float32)
    nb = q.shape[0]

    Wa, Wb, Wc, Wd = W1[0:36], W1[36:72], W1[72:108], W1[108:144]
    waug = np.empty((nb, 37, 80), dtype=np.float32)
    waug[:, 0:36, :] = (Wb - Wc)[None, :, :] + q[:, :, None] * Wd[None, :, :]
    waug[:, 36, :] = q @ (Wa + Wc) + b1[None, :]

    ubaug = np.empty((nb, 200, 37), dtype=np.float32)
    ubaug[:, :, 0:36] = ub
    ubaug[:, :, 36] = 1.0

    # sigmoid -> tanh fold: h = 0.5 + 0.5*t with t = tanh(pre/2)
    w2f = 0.5 * W2                                   # device mm2 weights
    b2f = 0.5 * (b2 + 0.5 * W2.sum(axis=0))          # ACT bias (scale=0.5 applied)
    w3f = 0.5 * W3
    b3f = float(b3[0] + 0.5 * W3.sum())

    w2p = np.zeros((80, 64), dtype=np.float32)
    w2p[:, 0:40] = w2f
    w3d0 = np.zeros((104, 32), dtype=np.float32)
    w3d0[0:40, 0] = w3f[:, 0]
    w3d1 = np.zeros((104, 32), dtype=np.float32)
    w3d1[64:104, 0] = w3f[:, 0]
    b2c = np.zeros((128, 1), dtype=np.float32)
    b2c[0:40, 0] = b2f
    b2c[64:104, 0] = b2f
    b3c = np.full((128, 1), b3f, dtype=np.float32)

    n_cores = nb // bc
    in_maps = []
    for c in range(n_cores):
        sl = slice(bc * c, bc * (c + 1))
        in_maps.append({
            "ubaug": np.ascontiguousarray(ubaug[sl].reshape(bc * 200, 37)),
            "waug": np.ascontiguousarray(waug[sl]),
            "lens": lens[sl].astype(np.float32).reshape(bc, 1),
            "w2": w2p, "w3d0": w3d0, "w3d1": w3d1, "b2c": b2c, "b3c": b3c,
        })
    return in_maps


_NC_CACHE = {}


def get_module(bc, pb):
    key = (bc, pb)
    if key not in _NC_CACHE:
        _NC_CACHE[key] = build_module(bc, pb)
    return _NC_CACHE[key]


def kernel(query_ad, user_behavior, user_behavior_length,
           W1, b1, W2, b2, W3, b3, trace=False):
    bc = B // N_CORES
    nc = get_module(bc, 64)
    in_maps = host_prep(query_ad, user_behavior, user_behavior_length,
                        W1, b1, W2, b2, W3, b3, bc)
    res = run_bass_kernel_spmd(nc, in_maps, core_ids=list(range(N_CORES)),
                               trace=trace)
    outs = [res.results[c]["out"] for c in range(N_CORES)]
    full = np.concatenate(outs, axis=0).reshape(B, 1, 36)
    if trace:
        kernel.last_result = res
    return full



# revision 5
# speedup vs baseline: 2.3039x; 2.3039x over previous
"""DIN attention layer kernel for Trainium2 (8 NeuronCores, data-parallel batch).

Reference math per (b,t):
  x  = concat([q, ub, q-ub, q*ub], -1)             # [B,T,144]
  h1 = sigmoid(x @ W1 + b1)                        # [B,T,80]
  h2 = sigmoid(h1 @ W2 + b2)                       # [B,T,40]
  s  = h2 @ W3 + b3                                # [B,T,1]
  w  = softmax(s.T * mask)                         # [B,1,T]  (multiplicative mask)
  out = w @ ub                                     # [B,1,36]

Design (v2, bf16):
  * All matmuls in bf16 (1 cycle/column vs 4 for fp32); accumulation fp32.
  * Host pre-transposes ub -> ubT [37, rows] (with a ones row) so mm1's rhs
    loads are large contiguous DMAs; no on-chip transposes anywhere.
  * x@W1 fold: x@W1+b1 = ubT.T@((Wb-Wc)+diag(q_b)Wd) + (q_b(Wa+Wc)+b1);
    host bakes per-batch lhsT [37,80] (36 fold rows + 1 bias row).
  * sigmoid(x)=0.5+0.5*tanh(x/2): tanh+exp share one ACT table set; the
    0.5/0.5 affine folds into the next layer's weights/biases.
  * mm2 packs two 512-col chunks into PSUM partitions [0:40] and [64:104].
  * mm3 uses sparse-column lhsT variants so 20 consecutive 512-row chunks
    accumulate into contiguous psum partitions 0..19 (one matmul serves a
    chunk pair); scores copy out in [20,512] blocks and roundtrip DRAM to
    land batch-major [128,200] for softmax.
  * Weighted sum w@ub on DVE: natural-layout ub [128 batches, 200*36] times
    softmax weights broadcast along features, then a strided reduce over t.
"""

from contextlib import ExitStack

import numpy as np
import ml_dtypes

import concourse.bass as bass
import concourse.bacc as bacc
import concourse.tile as tile
from concourse import mybir
from concourse.bass_utils import run_bass_kernel_spmd

B, T, E = 4096, 200, 36
N_CORES = 8
F32 = mybir.dt.float32
BF16 = mybir.dt.bfloat16
NPBF16 = ml_dtypes.bfloat16
AF = mybir.ActivationFunctionType


def build_module(bc=512):
    rows = bc * T                  # 102400
    n_slab = bc // 32              # 16 slabs of 32 batches
    slab_c = 32 * T                # 6400 cols per slab
    n_t = rows // 1024             # 100 mm1 psum tiles (1024 cols each)
    n_q = bc // 128                # 4 softmax tiles of 128 batches
    q_rows = 128 * T               # 25600 rows per softmax tile
    GP = 10                        # h2 tiles (chunk pairs) per mm3 psum group
    g_rows = GP * 1024             # 10240 rows per mm3 group

    nc = bacc.Bacc(
        "TRN2", target_bir_lowering=False, debug=False,
        enable_asserts=False, num_devices=N_CORES,
    )

    ubt_d = nc.dram_tensor("ubt", [37, rows], BF16, kind="ExternalInput").ap()
    ub_d = nc.dram_tensor("ubn", [rows, 36], BF16, kind="ExternalInput").ap()
    waug_d = nc.dram_tensor("waug", [37, bc * 80], BF16, kind="ExternalInput").ap()
    w2_d = nc.dram_tensor("w2t", [80, 40], BF16, kind="ExternalInput").ap()
    w3_d = nc.dram_tensor("w3v", [104, GP * 2 * GP], BF16,
                          kind="ExternalInput").ap()
    b2_d = nc.dram_tensor("b2c", [128, 1], F32, kind="ExternalInput").ap()
    b3_d = nc.dram_tensor("b3c", [2 * GP, 1], F32, kind="ExternalInput").ap()
    lens_d = nc.dram_tensor("lens2", [bc], F32, kind="ExternalInput").ap()
    iota_d = nc.dram_tensor("iota", [T], BF16, kind="ExternalInput").ap()
    out_d = nc.dram_tensor("out", [bc, 36], F32, kind="ExternalOutput").ap()
    scb_d = nc.dram_tensor("scb", [rows], BF16, kind="Internal").ap()

    with tile.TileContext(nc) as tc, ExitStack() as es:
        cpool = es.enter_context(tc.tile_pool(name="consts", bufs=1))
        h1p = es.enter_context(tc.tile_pool(name="h1p", bufs=4))
        h2p = es.enter_context(tc.tile_pool(name="h2p", bufs=4))
        scp = es.enter_context(tc.tile_pool(name="scp", bufs=3))
        smp = es.enter_context(tc.tile_pool(name="smp", bufs=2))
        ubgp = es.enter_context(tc.tile_pool(name="ubgp", bufs=2))
        wubp = es.enter_context(tc.tile_pool(name="wubp", bufs=2))
        rqp = es.enter_context(tc.tile_pool(name="rqp", bufs=6))
        outp = es.enter_context(tc.tile_pool(name="outp", bufs=2))
        m1p = es.enter_context(tc.tile_pool(name="m1p", bufs=2, space="PSUM"))
        m2p = es.enter_context(tc.tile_pool(name="m2p", bufs=2, space="PSUM"))
        m3p = es.enter_context(tc.tile_pool(name="m3p", bufs=2, space="PSUM"))

        # ---- constants ----
        w2t = cpool.tile([80, 40], BF16)
        nc.sync.dma_start(out=w2t, in_=w2_d)
        w3v = cpool.tile([104, GP, 2 * GP], BF16)
        nc.sync.dma_start(out=w3v, in_=w3_d)
        b2c = cpool.tile([128, 1], F32)
        nc.sync.dma_start(out=b2c, in_=b2_d)
        b3c = cpool.tile([2 * GP, 1], F32)
        nc.sync.dma_start(out=b3c, in_=b3_d)
        lens_sb = cpool.tile([128, n_q], F32)
        nc.sync.dma_start(
            out=lens_sb,
            in_=bass.AP(tensor=lens_d.tensor, offset=lens_d.offset,
                        ap=[[n_q, 128], [1, n_q]]))
        iota_t = cpool.tile([128, T], BF16)
        nc.sync.dma_start(
            out=iota_t,
            in_=bass.AP(tensor=iota_d.tensor, offset=iota_d.offset,
                        ap=[[0, 128], [1, T]]))

        # manually rotated (4-deep) slab buffers
        ubq2 = cpool.tile([37, 4, slab_c], BF16)     # rows 0-35 ubT, row 36 ones
        waug2 = cpool.tile([37, 4, 32 * 80], BF16)   # per-batch folded W1

        def emit_slab_load(s):
            slot = s % 4
            nc.sync.dma_start(
                out=ubq2[:, slot, :],
                in_=bass.AP(tensor=ubt_d.tensor,
                            offset=ubt_d.offset + slab_c * s,
                            ap=[[rows, 37], [1, slab_c]]))
            nc.sync.dma_start(
                out=waug2[:, slot, :],
                in_=bass.AP(tensor=waug_d.tensor,
                            offset=waug_d.offset + 32 * 80 * s,
                            ap=[[bc * 80, 37], [1, 32 * 80]]))

        def emit_ubg_load(q):
            ubg = ubgp.tile([128, T * 36], BF16, tag="ubg", name=f"ubg{q}")
            nc.scalar.dma_start(
                out=ubg,
                in_=bass.AP(tensor=ub_d.tensor,
                            offset=ub_d.offset + q_rows * 36 * q,
                            ap=[[T * 36, 128], [1, T * 36]]))
            return ubg

        def emit_softmax(q):
            sc_t = smp.tile([128, T], BF16, tag="sc")
            nc.scalar.dma_start(
                out=sc_t,
                in_=bass.AP(tensor=scb_d.tensor,
                            offset=scb_d.offset + q_rows * q,
                            ap=[[T, 128], [1, T]]))
            mask = smp.tile([128, T], BF16, tag="mask")
            nc.vector.tensor_scalar(
                out=mask, in0=iota_t, scalar1=lens_sb[:, q:q + 1],
                scalar2=None, op0=mybir.AluOpType.is_lt)
            masked = smp.tile([128, T], BF16, tag="masked")
            nc.vector.tensor_mul(masked, sc_t, mask)
            negmax = smp.tile([128, 1], F32, tag="negmax")
            nc.vector.tensor_reduce(
                out=negmax, in_=masked, axis=mybir.AxisListType.X,
                op=mybir.AluOpType.max, negate=True)
            ew = smp.tile([128, T], BF16, tag="ew")
            sumexp = smp.tile([128, 1], F32, tag="sumexp")
            nc.scalar.activation(
                out=ew, in_=masked, func=AF.Exp,
                bias=negmax, accum_out=sumexp)
            rz = smp.tile([128, 1], F32, tag="rz")
            nc.vector.reciprocal(rz, sumexp)
            wt = smp.tile([128, T], BF16, tag="wt", name=f"wt{q}")
            nc.vector.tensor_scalar_mul(wt, ew, rz)
            return wt

        mv_parts = {}

        def emit_mv_quarter(q, u, ubg, wt):
            # wub = ub * w over a 50-t slice; reduce over t -> [128, 36]
            t0 = 50 * u
            wub = wubp.tile([128, 50 * 36], BF16, tag="wub")
            nc.vector.tensor_mul(
                wub.rearrange("p (t e) -> p t e", e=36),
                ubg[:, t0 * 36:(t0 + 50) * 36].rearrange(
                    "p (t e) -> p t e", e=36),
                wt[:, t0:t0 + 50].unsqueeze(2).broadcast_to([128, 50, 36]))
            rq = rqp.tile([128, 36], F32, tag=f"rq{u}")
            nc.vector.tensor_reduce(
                out=rq, in_=wub.rearrange("p (t e) -> p e t", e=36),
                axis=mybir.AxisListType.X, op=mybir.AluOpType.add)
            mv_parts.setdefault(q, []).append(rq)
            if u == 3:
                r = mv_parts.pop(q)
                s01 = rqp.tile([128, 36], F32, tag="s01")
                nc.vector.tensor_add(s01, r[0], r[1])
                s23 = rqp.tile([128, 36], F32, tag="s23")
                nc.vector.tensor_add(s23, r[2], r[3])
                ot = outp.tile([128, 36], F32, tag="ot")
                nc.vector.tensor_add(ot, s01, s23)
                nc.sync.dma_start(out=out_d[128 * q:128 * (q + 1), :], in_=ot)

        # score-store trigger tiles for each softmax window, then the mv
        # quarters of window q interleaved over the following tiles
        trig_t = {}
        for q in range(n_q):
            g = -(-q_rows * (q + 1) // g_rows)    # ceil
            trig_t[q] = g * GP - 1
        mv_sched = {}
        for q in range(n_q):
            for u in range(4):
                tt = trig_t[q] + 4 + 6 * u
                if tt < n_t:
                    mv_sched[tt] = (q, u)

        next_slab = 0
        next_q = 0
        pend = {}     # q -> [ubg, wt]
        mv_done = set()

        for t in range(n_t):
            c_lo, c_hi = 1024 * t, 1024 * t + 1024
            while next_slab < n_slab and slab_c * next_slab < c_hi:
                emit_slab_load(next_slab)
                next_slab += 1
            if t % (n_t // n_q) == 2:
                q = t // (n_t // n_q)
                if q < n_q:
                    pend[q] = [emit_ubg_load(q), None]

            m1 = m1p.tile([80, 1024], F32, tag="m1")
            b0, b1_ = c_lo // T, (c_hi - 1) // T
            for b in range(b0, b1_ + 1):
                c0 = max(T * b, c_lo)
                c1 = min(T * b + T, c_hi)
                s = b // 32
                slot = s % 4
                nc.tensor.matmul(
                    m1[0:80, c0 - c_lo:c1 - c_lo],
                    waug2[:, slot, 80 * (b % 32):80 * (b % 32) + 80],
                    ubq2[:, slot, c0 - slab_c * s:c1 - slab_c * s],
                    start=True, stop=True)

            h1t = h1p.tile([80, 1024], BF16, tag="h1")
            nc.scalar.activation(out=h1t, in_=m1, func=AF.Tanh, scale=0.5)

            m2 = m2p.tile([128, 512], F32, tag="m2")
            nc.tensor.matmul(m2[0:40, :], w2t, h1t[:, 0:512],
                             start=True, stop=True)
            nc.tensor.matmul(m2[64:104, :], w2t, h1t[:, 512:1024],
                             start=True, stop=True)

            h2t = h2p.tile([128, 512], BF16, tag="h2")
            nc.scalar.activation(out=h2t, in_=m2, func=AF.Tanh,
                                 bias=b2c, scale=0.5)

            # mm3: one matmul per h2 tile scores chunk-pair (2t, 2t+1) into
            # psum rows (2j, 2j+1) of group g = t // GP, j = t % GP
            j = t % GP
            if j == 0:
                m3 = m3p.tile([2 * GP, 512], F32, tag="m3")
            nc.tensor.matmul(
                m3, w3v[:, j, :], h2t[0:104, :],
                start=(j == 0), stop=(j == GP - 1))
            if j == GP - 1:
                g = t // GP
                sct = scp.tile([2 * GP, 512], BF16, tag="sct")
                nc.vector.tensor_scalar_add(sct, m3, b3c)
                nc.sync.dma_start(
                    out=bass.AP(tensor=scb_d.tensor,
                                offset=scb_d.offset + g_rows * g,
                                ap=[[512, 2 * GP], [1, 512]]),
                    in_=sct)
                while next_q < n_q and g_rows * (g + 1) >= q_rows * (next_q + 1):
                    pend[next_q][1] = emit_softmax(next_q)
                    next_q += 1

            if t in mv_sched:
                q, u = mv_sched[t]
                emit_mv_quarter(q, u, pend[q][0], pend[q][1])
                mv_done.add((q, u))

        # tail: any quarters not interleaved above
        for q in range(n_q):
            for u in range(4):
                if (q, u) not in mv_done:
                    emit_mv_quarter(q, u, pend[q][0], pend[q][1])

    nc.compile()
    return nc


def host_prep(query_ad, user_behavior, user_behavior_length,
              W1, b1, W2, b2, W3, b3, bc):
    GP = 10
    q = np.asarray(query_ad, dtype=np.float32)
    ub = np.asarray(user_behavior, dtype=np.float32)
    lens = np.asarray(user_behavior_length)
    W1 = np.asarray(W1, dtype=np.float32)
    b1 = np.asarray(b1, dtype=np.float32)
    W2 = np.asarray(W2, dtype=np.float32)
    b2 = np.asarray(b2, dtype=np.float32)
    W3 = np.asarray(W3, dtype=np.float32)
    b3 = np.asarray(b3, dtype=np.float32)
    nb = q.shape[0]
    n_cores = nb // bc

    Wa, Wb, Wc, Wd = W1[0:36], W1[36:72], W1[72:108], W1[108:144]
    # per-batch folded mm1 weights: [nb, 37, 80]
    waug = np.empty((nb, 37, 80), dtype=np.float32)
    waug[:, 0:36, :] = (Wb - Wc)[None, :, :] + q[:, :, None] * Wd[None, :, :]
    waug[:, 36, :] = q @ (Wa + Wc) + b1[None, :]

    # sigmoid -> tanh folds (see module docstring)
    w2f = (0.5 * W2).astype(NPBF16)                        # [80,40]
    b2f = 0.5 * (b2 + 0.5 * W2.sum(axis=0))                # [40]
    w3f = 0.5 * W3[:, 0]                                   # [40]
    b3f = float(b3[0] + 0.5 * W3.sum())

    b2c = np.zeros((128, 1), dtype=np.float32)
    b2c[0:40, 0] = b2f
    b2c[64:104, 0] = b2f
    # mm3 lhsT variants: variant j ([104, 2*GP]) has w3 in column 2j
    # (h2 rows 0-39, even chunk of the pair) and column 2j+1 (rows 64-103)
    w3v = np.zeros((104, GP, 2 * GP), dtype=np.float32)
    for j in range(GP):
        w3v[0:40, j, 2 * j] = w3f
        w3v[64:104, j, 2 * j + 1] = w3f
    w3v = w3v.reshape(104, GP * 2 * GP).astype(NPBF16)
    b3c = np.full((2 * GP, 1), b3f, dtype=np.float32)
    iota = np.arange(T, dtype=np.float32).astype(NPBF16)

    ub16 = ub.astype(NPBF16)                               # [nb, T, 36]

    in_maps = []
    for c in range(n_cores):
        sl = slice(bc * c, bc * (c + 1))
        ubc = ub16[sl].reshape(bc * T, 36)
        ubt = np.empty((37, bc * T), dtype=NPBF16)
        ubt[0:36] = ubc.T
        ubt[36] = 1.0
        in_maps.append({
            "ubt": ubt,
            "ubn": np.ascontiguousarray(ubc),
            "waug": np.ascontiguousarray(
                waug[sl].transpose(1, 0, 2).reshape(37, bc * 80)
            ).astype(NPBF16),
            "w2t": w2f, "w3v": w3v, "b2c": b2c, "b3c": b3c,
            "lens2": np.ascontiguousarray(
                lens[sl].astype(np.float32).reshape(bc // 128, 128).T
            ).reshape(bc),
            "iota": iota,
        })
    return in_maps


_NC_CACHE = {}


def get_module(bc):
    if bc not in _NC_CACHE:
        _NC_CACHE[bc] = build_module(bc)
    return _NC_CACHE[bc]


def kernel(query_ad, user_behavior, user_behavior_length,
           W1, b1, W2, b2, W3, b3, trace=False):
    bc = B // N_CORES
    nc = get_module(bc)
    in_maps = host_prep(query_ad, user_behavior, user_behavior_length,
                        W1, b1, W2, b2, W3, b3, bc)
    res = run_bass_kernel_spmd(nc, in_maps, core_ids=list(range(N_CORES)),
                               trace=trace)
    outs = [res.results[c]["out"] for c in range(N_CORES)]
    full = np.concatenate(outs, axis=0).reshape(B, 1, 36)
    if trace:
        kernel.last_result = res
    return full


# revision 7
# speedup vs baseline: 2.3490x; 1.0196x over previous
"""DIN attention layer kernel for Trainium2 (8 NeuronCores, data-parallel batch).

Reference math per (b,t):
  x  = concat([q, ub, q-ub, q*ub], -1)             # [B,T,144]
  h1 = sigmoid(x @ W1 + b1)                        # [B,T,80]
  h2 = sigmoid(h1 @ W2 + b2)                       # [B,T,40]
  s  = h2 @ W3 + b3                                # [B,T,1]
  w  = softmax(s.T * mask)                         # [B,1,T]  (multiplicative mask)
  out = w @ ub                                     # [B,1,36]

Design (v2, bf16):
  * All matmuls in bf16 (1 cycle/column vs 4 for fp32); accumulation fp32.
  * Host pre-transposes ub -> ubT [37, rows] (with a ones row) so mm1's rhs
    loads are large contiguous DMAs; no on-chip transposes anywhere.
  * x@W1 fold: x@W1+b1 = ubT.T@((Wb-Wc)+diag(q_b)Wd) + (q_b(Wa+Wc)+b1);
    host bakes per-batch lhsT [37,80] (36 fold rows + 1 bias row).
  * sigmoid(x)=0.5+0.5*tanh(x/2): tanh+exp share one ACT table set; the
    0.5/0.5 affine folds into the next layer's weights/biases.
  * mm2 packs two 512-col chunks into PSUM partitions [0:40] and [64:104].
  * mm3 uses sparse-column lhsT variants so 20 consecutive 512-row chunks
    accumulate into contiguous psum partitions 0..19 (one matmul serves a
    chunk pair); scores copy out in [20,512] blocks and roundtrip DRAM to
    land batch-major [128,200] for softmax.
  * Weighted sum w@ub on DVE: natural-layout ub [128 batches, 200*36] times
    softmax weights broadcast along features, then a strided reduce over t.
"""

from contextlib import ExitStack

import numpy as np
import ml_dtypes

import concourse.bass as bass
import concourse.bacc as bacc
import concourse.tile as tile
from concourse import mybir
from concourse.bass_utils import run_bass_kernel_spmd

B, T, E = 4096, 200, 36
N_CORES = 8
F32 = mybir.dt.float32
BF16 = mybir.dt.bfloat16
NPBF16 = ml_dtypes.bfloat16
AF = mybir.ActivationFunctionType


def build_module(bc=512):
    rows = bc * T                  # 102400
    n_slab = bc // 32              # 16 slabs of 32 batches
    slab_c = 32 * T                # 6400 cols per slab
    n_t = rows // 1024             # 100 mm1 psum tiles (1024 cols each)
    n_q = bc // 128                # 4 softmax tiles of 128 batches
    q_rows = 128 * T               # 25600 rows per softmax tile
    GP = 10                        # h2 tiles (chunk pairs) per mm3 psum group
    g_rows = GP * 1024             # 10240 rows per mm3 group

    nc = bacc.Bacc(
        "TRN2", target_bir_lowering=False, debug=False,
        enable_asserts=False, num_devices=N_CORES,
    )

    ubt_d = nc.dram_tensor("ubt", [37, rows], BF16, kind="ExternalInput").ap()
    ub_d = nc.dram_tensor("ubn", [rows, 36], BF16, kind="ExternalInput").ap()
    waug_d = nc.dram_tensor("waug", [37, bc * 80], BF16, kind="ExternalInput").ap()
    w2_d = nc.dram_tensor("w2t", [80, 40], BF16, kind="ExternalInput").ap()
    w3_d = nc.dram_tensor("w3v", [104, GP * 2 * GP], BF16,
                          kind="ExternalInput").ap()
    b2_d = nc.dram_tensor("b2c", [128, 1], F32, kind="ExternalInput").ap()
    b3_d = nc.dram_tensor("b3c", [2 * GP, 1], F32, kind="ExternalInput").ap()
    lens_d = nc.dram_tensor("lens2", [bc], F32, kind="ExternalInput").ap()
    iota_d = nc.dram_tensor("iota", [T], BF16, kind="ExternalInput").ap()
    out_d = nc.dram_tensor("out", [bc, 36], F32, kind="ExternalOutput").ap()
    scb_d = nc.dram_tensor("scb", [rows], BF16, kind="Internal").ap()

    with tile.TileContext(nc) as tc, ExitStack() as es:
        cpool = es.enter_context(tc.tile_pool(name="consts", bufs=1))
        h1p = es.enter_context(tc.tile_pool(name="h1p", bufs=4))
        h2p = es.enter_context(tc.tile_pool(name="h2p", bufs=4))
        scp = es.enter_context(tc.tile_pool(name="scp", bufs=3))
        smp = es.enter_context(tc.tile_pool(name="smp", bufs=2))
        ubgp = es.enter_context(tc.tile_pool(name="ubgp", bufs=2))
        wubp = es.enter_context(tc.tile_pool(name="wubp", bufs=2))
        rqp = es.enter_context(tc.tile_pool(name="rqp", bufs=6))
        outp = es.enter_context(tc.tile_pool(name="outp", bufs=2))
        m1p = es.enter_context(tc.tile_pool(name="m1p", bufs=2, space="PSUM"))
        m2p = es.enter_context(tc.tile_pool(name="m2p", bufs=2, space="PSUM"))
        m3p = es.enter_context(tc.tile_pool(name="m3p", bufs=2, space="PSUM"))

        # ---- constants ----
        w2t = cpool.tile([80, 40], BF16)
        nc.sync.dma_start(out=w2t, in_=w2_d)
        w3v = cpool.tile([104, GP, 2 * GP], BF16)
        nc.sync.dma_start(out=w3v, in_=w3_d)
        b2c = cpool.tile([128, 1], F32)
        nc.sync.dma_start(out=b2c, in_=b2_d)
        b3c = cpool.tile([2 * GP, 1], F32)
        nc.sync.dma_start(out=b3c, in_=b3_d)
        lens_sb = cpool.tile([128, n_q], F32)
        nc.sync.dma_start(
            out=lens_sb,
            in_=bass.AP(tensor=lens_d.tensor, offset=lens_d.offset,
                        ap=[[n_q, 128], [1, n_q]]))
        iota_t = cpool.tile([128, T], BF16)
        nc.sync.dma_start(
            out=iota_t,
            in_=bass.AP(tensor=iota_d.tensor, offset=iota_d.offset,
                        ap=[[0, 128], [1, T]]))

        # manually rotated (4-deep) slab buffers
        ubq2 = cpool.tile([37, 4, slab_c], BF16)     # rows 0-35 ubT, row 36 ones
        waug2 = cpool.tile([37, 4, 32 * 80], BF16)   # per-batch folded W1

        def emit_slab_load(s):
            slot = s % 4
            nc.sync.dma_start(
                out=ubq2[:, slot, :],
                in_=bass.AP(tensor=ubt_d.tensor,
                            offset=ubt_d.offset + slab_c * s,
                            ap=[[rows, 37], [1, slab_c]]))
            nc.sync.dma_start(
                out=waug2[:, slot, :],
                in_=bass.AP(tensor=waug_d.tensor,
                            offset=waug_d.offset + 32 * 80 * s,
                            ap=[[bc * 80, 37], [1, 32 * 80]]))

        def emit_ubg_load(q):
            ubg = ubgp.tile([128, T * 36], BF16, tag="ubg", name=f"ubg{q}")
            nc.gpsimd.dma_start(
                out=ubg,
                in_=bass.AP(tensor=ub_d.tensor,
                            offset=ub_d.offset + q_rows * 36 * q,
                            ap=[[T * 36, 128], [1, T * 36]]))
            return ubg

        def emit_softmax(q):
            sc_t = smp.tile([128, T], BF16, tag="sc")
            nc.gpsimd.dma_start(
                out=sc_t,
                in_=bass.AP(tensor=scb_d.tensor,
                            offset=scb_d.offset + q_rows * q,
                            ap=[[T, 128], [1, T]]))
            mask = smp.tile([128, T], BF16, tag="mask")
            nc.vector.tensor_scalar(
                out=mask, in0=iota_t, scalar1=lens_sb[:, q:q + 1],
                scalar2=None, op0=mybir.AluOpType.is_lt)
            masked = smp.tile([128, T], BF16, tag="masked")
            nc.vector.tensor_mul(masked, sc_t, mask)
            negmax = smp.tile([128, 1], F32, tag="negmax")
            nc.vector.tensor_reduce(
                out=negmax, in_=masked, axis=mybir.AxisListType.X,
                op=mybir.AluOpType.max, negate=True)
            ew = smp.tile([128, T], BF16, tag="ew")
            sumexp = smp.tile([128, 1], F32, tag="sumexp")
            nc.scalar.activation(
                out=ew, in_=masked, func=AF.Exp,
                bias=negmax, accum_out=sumexp)
            rz = smp.tile([128, 1], F32, tag="rz")
            nc.vector.reciprocal(rz, sumexp)
            wt = smp.tile([128, T], BF16, tag="wt", name=f"wt{q}")
            nc.vector.tensor_scalar_mul(wt, ew, rz)
            return wt

        mv_parts = {}

        def emit_mv_quarter(q, u, ubg, wt):
            # wub = ub * w over a 50-t slice; reduce over t -> [128, 36]
            t0 = 50 * u
            wub = wubp.tile([128, 50 * 36], BF16, tag="wub")
            nc.vector.tensor_mul(
                wub.rearrange("p (t e) -> p t e", e=36),
                ubg[:, t0 * 36:(t0 + 50) * 36].rearrange(
                    "p (t e) -> p t e", e=36),
                wt[:, t0:t0 + 50].unsqueeze(2).broadcast_to([128, 50, 36]))
            rq = rqp.tile([128, 36], F32, tag=f"rq{u}")
            nc.vector.tensor_reduce(
                out=rq, in_=wub.rearrange("p (t e) -> p e t", e=36),
                axis=mybir.AxisListType.X, op=mybir.AluOpType.add)
            mv_parts.setdefault(q, []).append(rq)
            if u == 3:
                r = mv_parts.pop(q)
                s01 = rqp.tile([128, 36], F32, tag="s01")
                nc.vector.tensor_add(s01, r[0], r[1])
                s23 = rqp.tile([128, 36], F32, tag="s23")
                nc.vector.tensor_add(s23, r[2], r[3])
                ot = outp.tile([128, 36], F32, tag="ot")
                nc.vector.tensor_add(ot, s01, s23)
                nc.sync.dma_start(out=out_d[128 * q:128 * (q + 1), :], in_=ot)

        # score-store trigger tiles for each softmax window, then the mv
        # quarters of window q interleaved over the following tiles
        trig_t = {}
        for q in range(n_q):
            g = -(-q_rows * (q + 1) // g_rows)    # ceil
            trig_t[q] = g * GP - 1
        mv_sched = {}
        for q in range(n_q):
            for u in range(4):
                tt = trig_t[q] + 4 + 6 * u
                if tt < n_t:
                    mv_sched[tt] = (q, u)

        next_slab = 0
        next_q = 0
        pend = {}     # q -> [ubg, wt]
        mv_done = set()

        for t in range(n_t):
            c_lo, c_hi = 1024 * t, 1024 * t + 1024
            while next_slab < n_slab and slab_c * next_slab < c_hi:
                emit_slab_load(next_slab)
                next_slab += 1
            if t % (n_t // n_q) == 2:
                q = t // (n_t // n_q)
                if q < n_q:
                    pend[q] = [emit_ubg_load(q), None]

            m1 = m1p.tile([80, 1024], F32, tag="m1")
            b0, b1_ = c_lo // T, (c_hi - 1) // T
            for b in range(b0, b1_ + 1):
                c0 = max(T * b, c_lo)
                c1 = min(T * b + T, c_hi)
                s = b // 32
                slot = s % 4
                nc.tensor.matmul(
                    m1[0:80, c0 - c_lo:c1 - c_lo],
                    waug2[:, slot, 80 * (b % 32):80 * (b % 32) + 80],
                    ubq2[:, slot, c0 - slab_c * s:c1 - slab_c * s],
                    start=True, stop=True)

            h1t = h1p.tile([80, 1024], BF16, tag="h1")
            nc.scalar.activation(out=h1t, in_=m1, func=AF.Tanh, scale=0.5)

            m2 = m2p.tile([128, 512], F32, tag="m2")
            nc.tensor.matmul(m2[0:40, :], w2t, h1t[:, 0:512],
                             start=True, stop=True)
            nc.tensor.matmul(m2[64:104, :], w2t, h1t[:, 512:1024],
                             start=True, stop=True)

            h2t = h2p.tile([128, 512], BF16, tag="h2")
            nc.scalar.activation(out=h2t, in_=m2, func=AF.Tanh,
                                 bias=b2c, scale=0.5)

            # mm3: one matmul per h2 tile scores chunk-pair (2t, 2t+1) into
            # psum rows (2j, 2j+1) of group g = t // GP, j = t % GP
            j = t % GP
            if j == 0:
                m3 = m3p.tile([2 * GP, 512], F32, tag="m3")
            nc.tensor.matmul(
                m3, w3v[:, j, :], h2t[0:104, :],
                start=(j == 0), stop=(j == GP - 1))
            if j == GP - 1:
                g = t // GP
                sct = scp.tile([2 * GP, 512], BF16, tag="sct")
                nc.vector.tensor_scalar_add(sct, m3, b3c)
                nc.sync.dma_start(
                    out=bass.AP(tensor=scb_d.tensor,
                                offset=scb_d.offset + g_rows * g,
                                ap=[[512, 2 * GP], [1, 512]]),
                    in_=sct)
                while next_q < n_q and g_rows * (g + 1) >= q_rows * (next_q + 1):
                    pend[next_q][1] = emit_softmax(next_q)
                    next_q += 1

            if t in mv_sched:
                q, u = mv_sched[t]
                emit_mv_quarter(q, u, pend[q][0], pend[q][1])
                mv_done.add((q, u))

        # tail: any quarters not interleaved above
        for q in range(n_q):
            for u in range(4):
                if (q, u) not in mv_done:
                    emit_mv_quarter(q, u, pend[q][0], pend[q][1])

    nc.compile()
    return nc


def host_prep(query_ad, user_behavior, user_behavior_length,
              W1, b1, W2, b2, W3, b3, bc):
    GP = 10
    q = np.asarray(query_ad, dtype=np.float32)
    ub = np.asarray(user_behavior, dtype=np.float32)
    lens = np.asarray(user_behavior_length)
    W1 = np.asarray(W1, dtype=np.float32)
    b1 = np.asarray(b1, dtype=np.float32)
    W2 = np.asarray(W2, dtype=np.float32)
    b2 = np.asarray(b2, dtype=np.float32)
    W3 = np.asarray(W3, dtype=np.float32)
    b3 = np.asarray(b3, dtype=np.float32)
    nb = q.shape[0]
    n_cores = nb // bc

    Wa, Wb, Wc, Wd = W1[0:36], W1[36:72], W1[72:108], W1[108:144]
    # per-batch folded mm1 weights: [nb, 37, 80]
    waug = np.empty((nb, 37, 80), dtype=np.float32)
    waug[:, 0:36, :] = (Wb - Wc)[None, :, :] + q[:, :, None] * Wd[None, :, :]
    waug[:, 36, :] = q @ (Wa + Wc) + b1[None, :]

    # sigmoid -> tanh folds (see module docstring)
    w2f = (0.5 * W2).astype(NPBF16)                        # [80,40]
    b2f = 0.5 * (b2 + 0.5 * W2.sum(axis=0))                # [40]
    w3f = 0.5 * W3[:, 0]                                   # [40]
    b3f = float(b3[0] + 0.5 * W3.sum())

    b2c = np.zeros((128, 1), dtype=np.float32)
    b2c[0:40, 0] = b2f
    b2c[64:104, 0] = b2f
    # mm3 lhsT variants: variant j ([104, 2*GP]) has w3 in column 2j
    # (h2 rows 0-39, even chunk of the pair) and column 2j+1 (rows 64-103)
    w3v = np.zeros((104, GP, 2 * GP), dtype=np.float32)
    for j in range(GP):
        w3v[0:40, j, 2 * j] = w3f
        w3v[64:104, j, 2 * j + 1] = w3f
    w3v = w3v.reshape(104, GP * 2 * GP).astype(NPBF16)
    b3c = np.full((2 * GP, 1), b3f, dtype=np.float32)
    iota = np.arange(T, dtype=np.float32).astype(NPBF16)

    ub16 = ub.astype(NPBF16)                               # [nb, T, 36]

    in_maps = []
    for c in range(n_cores):
        sl = slice(bc * c, bc * (c + 1))
        ubc = ub16[sl].reshape(bc * T, 36)
        ubt = np.empty((37, bc * T), dtype=NPBF16)
        ubt[0:36] = ubc.T
        ubt[36] = 1.0
        in_maps.append({
            "ubt": ubt,
            "ubn": np.ascontiguousarray(ubc),
            "waug": np.ascontiguousarray(
                waug[sl].transpose(1, 0, 2).reshape(37, bc * 80)
            ).astype(NPBF16),
            "w2t": w2f, "w3v": w3v, "b2c": b2c, "b3c": b3c,
            "lens2": np.ascontiguousarray(
                lens[sl].astype(np.float32).reshape(bc // 128, 128).T
            ).reshape(bc),
            "iota": iota,
        })
    return in_maps


_NC_CACHE = {}


def get_module(bc):
    if bc not in _NC_CACHE:
        _NC_CACHE[bc] = build_module(bc)
    return _NC_CACHE[bc]


def kernel(query_ad, user_behavior, user_behavior_length,
           W1, b1, W2, b2, W3, b3, trace=False):
    bc = B // N_CORES
    nc = get_module(bc)
    in_maps = host_prep(query_ad, user_behavior, user_behavior_length,
                        W1, b1, W2, b2, W3, b3, bc)
    res = run_bass_kernel_spmd(nc, in_maps, core_ids=list(range(N_CORES)),
                               trace=trace)
    outs = [res.results[c]["out"] for c in range(N_CORES)]
    full = np.concatenate(outs, axis=0).reshape(B, 1, 36)
    if trace:
        kernel.last_result = res
    return full


# revision 9
# speedup vs baseline: 2.4476x; 1.0419x over previous
"""DIN attention layer kernel for Trainium2 (8 NeuronCores, data-parallel batch).

Reference math per (b,t):
  x  = concat([q, ub, q-ub, q*ub], -1)             # [B,T,144]
  h1 = sigmoid(x @ W1 + b1)                        # [B,T,80]
  h2 = sigmoid(h1 @ W2 + b2)                       # [B,T,40]
  s  = h2 @ W3 + b3                                # [B,T,1]
  w  = softmax(s.T * mask)                         # [B,1,T]  (multiplicative mask)
  out = w @ ub                                     # [B,1,36]

Design (v2, bf16):
  * All matmuls in bf16 (1 cycle/column vs 4 for fp32); accumulation fp32.
  * Host pre-transposes ub -> ubT [37, rows] (with a ones row) so mm1's rhs
    loads are large contiguous DMAs; no on-chip transposes anywhere.
  * x@W1 fold: x@W1+b1 = ubT.T@((Wb-Wc)+diag(q_b)Wd) + (q_b(Wa+Wc)+b1);
    host bakes per-batch lhsT [37,80] (36 fold rows + 1 bias row).
  * sigmoid(x)=0.5+0.5*tanh(x/2): tanh+exp share one ACT table set; the
    0.5/0.5 affine folds into the next layer's weights/biases.
  * mm2 packs two 512-col chunks into PSUM partitions [0:40] and [64:104].
  * mm3 uses sparse-column lhsT variants so 20 consecutive 512-row chunks
    accumulate into contiguous psum partitions 0..19 (one matmul serves a
    chunk pair); scores copy out in [20,512] blocks and roundtrip DRAM to
    land batch-major [128,200] for softmax.
  * Weighted sum w@ub on DVE: natural-layout ub [128 batches, 200*36] times
    softmax weights broadcast along features, then a strided reduce over t.
"""

from contextlib import ExitStack

import numpy as np
import ml_dtypes

import concourse.bass as bass
import concourse.bacc as bacc
import concourse.tile as tile
from concourse import mybir
from concourse.bass_utils import run_bass_kernel_spmd

B, T, E = 4096, 200, 36
N_CORES = 8
F32 = mybir.dt.float32
BF16 = mybir.dt.bfloat16
NPBF16 = ml_dtypes.bfloat16
AF = mybir.ActivationFunctionType


def build_module(bc=512):
    rows = bc * T                  # 102400
    n_slab = bc // 32              # 16 slabs of 32 batches
    slab_c = 32 * T                # 6400 cols per slab
    n_t = rows // 1024             # 100 mm1 psum tiles (1024 cols each)
    n_q = bc // 128                # 4 softmax tiles of 128 batches
    q_rows = 128 * T               # 25600 rows per softmax tile
    GP = 10                        # h2 tiles (chunk pairs) per mm3 psum group
    g_rows = GP * 1024             # 10240 rows per mm3 group

    nc = bacc.Bacc(
        "TRN2", target_bir_lowering=False, debug=False,
        enable_asserts=False, num_devices=N_CORES,
    )

    ubt_d = nc.dram_tensor("ubt", [37, rows], BF16, kind="ExternalInput").ap()
    ub_d = nc.dram_tensor("ubn", [rows, 36], BF16, kind="ExternalInput").ap()
    waug_d = nc.dram_tensor("waug", [37, bc * 80], BF16, kind="ExternalInput").ap()
    w2_d = nc.dram_tensor("w2t", [80, 40], BF16, kind="ExternalInput").ap()
    w3_d = nc.dram_tensor("w3v", [104, GP * 2 * GP], BF16,
                          kind="ExternalInput").ap()
    b2_d = nc.dram_tensor("b2c", [128, 1], F32, kind="ExternalInput").ap()
    b3_d = nc.dram_tensor("b3c", [2 * GP, 1], F32, kind="ExternalInput").ap()
    lens_d = nc.dram_tensor("lens2", [bc], F32, kind="ExternalInput").ap()
    iota_d = nc.dram_tensor("iota", [T], BF16, kind="ExternalInput").ap()
    out_d = nc.dram_tensor("out", [bc, 36], F32, kind="ExternalOutput").ap()
    scb_d = nc.dram_tensor("scb", [rows], BF16, kind="Internal").ap()

    with tile.TileContext(nc) as tc, ExitStack() as es:
        cpool = es.enter_context(tc.tile_pool(name="consts", bufs=1))
        h1p = es.enter_context(tc.tile_pool(name="h1p", bufs=4))
        h2p = es.enter_context(tc.tile_pool(name="h2p", bufs=4))
        scp = es.enter_context(tc.tile_pool(name="scp", bufs=3))
        smp = es.enter_context(tc.tile_pool(name="smp", bufs=2))
        ubgp = es.enter_context(tc.tile_pool(name="ubgp", bufs=2))
        wubp = es.enter_context(tc.tile_pool(name="wubp", bufs=2))
        rqp = es.enter_context(tc.tile_pool(name="rqp", bufs=6))
        outp = es.enter_context(tc.tile_pool(name="outp", bufs=2))
        m1p = es.enter_context(tc.tile_pool(name="m1p", bufs=2, space="PSUM"))
        m2p = es.enter_context(tc.tile_pool(name="m2p", bufs=2, space="PSUM"))
        m3p = es.enter_context(tc.tile_pool(name="m3p", bufs=2, space="PSUM"))

        # ---- constants ----
        w2t = cpool.tile([80, 40], BF16)
        nc.sync.dma_start(out=w2t, in_=w2_d)
        w3v = cpool.tile([104, GP, 2 * GP], BF16)
        nc.sync.dma_start(out=w3v, in_=w3_d)
        b2c = cpool.tile([128, 1], F32)
        nc.sync.dma_start(out=b2c, in_=b2_d)
        b3c = cpool.tile([2 * GP, 1], F32)
        nc.sync.dma_start(out=b3c, in_=b3_d)
        lens_sb = cpool.tile([128, n_q], F32)
        nc.sync.dma_start(
            out=lens_sb,
            in_=bass.AP(tensor=lens_d.tensor, offset=lens_d.offset,
                        ap=[[n_q, 128], [1, n_q]]))
        iota_t = cpool.tile([128, T], BF16)
        nc.sync.dma_start(
            out=iota_t,
            in_=bass.AP(tensor=iota_d.tensor, offset=iota_d.offset,
                        ap=[[0, 128], [1, T]]))

        # manually rotated (6-deep) slab buffers
        NSLOT = 6
        ubq2 = cpool.tile([37, NSLOT, slab_c], BF16)  # rows 0-35 ubT, row 36 ones
        waug2 = cpool.tile([37, NSLOT, 32 * 80], BF16)  # per-batch folded W1

        def emit_slab_load(s):
            slot = s % NSLOT
            nc.sync.dma_start(
                out=ubq2[:, slot, :],
                in_=bass.AP(tensor=ubt_d.tensor,
                            offset=ubt_d.offset + slab_c * s,
                            ap=[[rows, 37], [1, slab_c]]))
            nc.sync.dma_start(
                out=waug2[:, slot, :],
                in_=bass.AP(tensor=waug_d.tensor,
                            offset=waug_d.offset + 32 * 80 * s,
                            ap=[[bc * 80, 37], [1, 32 * 80]]))

        def emit_ubg_load(q):
            ubg = ubgp.tile([128, T * 36], BF16, tag="ubg", name=f"ubg{q}")
            nc.gpsimd.dma_start(
                out=ubg,
                in_=bass.AP(tensor=ub_d.tensor,
                            offset=ub_d.offset + q_rows * 36 * q,
                            ap=[[T * 36, 128], [1, T * 36]]))
            return ubg

        def emit_softmax(q):
            sc_t = smp.tile([128, T], BF16, tag="sc")
            nc.gpsimd.dma_start(
                out=sc_t,
                in_=bass.AP(tensor=scb_d.tensor,
                            offset=scb_d.offset + q_rows * q,
                            ap=[[T, 128], [1, T]]))
            mask = smp.tile([128, T], BF16, tag="mask")
            nc.vector.tensor_scalar(
                out=mask, in0=iota_t, scalar1=lens_sb[:, q:q + 1],
                scalar2=None, op0=mybir.AluOpType.is_lt)
            masked = smp.tile([128, T], BF16, tag="masked")
            nc.vector.tensor_mul(masked, sc_t, mask)
            negmax = smp.tile([128, 1], F32, tag="negmax")
            nc.vector.tensor_reduce(
                out=negmax, in_=masked, axis=mybir.AxisListType.X,
                op=mybir.AluOpType.max, negate=True)
            ew = smp.tile([128, T], BF16, tag="ew")
            sumexp = smp.tile([128, 1], F32, tag="sumexp")
            nc.scalar.activation(
                out=ew, in_=masked, func=AF.Exp,
                bias=negmax, accum_out=sumexp)
            rz = smp.tile([128, 1], F32, tag="rz")
            nc.vector.reciprocal(rz, sumexp)
            wt = smp.tile([128, T], BF16, tag="wt", name=f"wt{q}")
            nc.vector.tensor_scalar_mul(wt, ew, rz)
            return wt

        mv_parts = {}

        def emit_mv_quarter(q, u, ubg, wt):
            # wub = ub * w over a 50-t slice; reduce over t -> [128, 36]
            t0 = 50 * u
            wub = wubp.tile([128, 50 * 36], BF16, tag="wub")
            nc.vector.tensor_mul(
                wub.rearrange("p (t e) -> p t e", e=36),
                ubg[:, t0 * 36:(t0 + 50) * 36].rearrange(
                    "p (t e) -> p t e", e=36),
                wt[:, t0:t0 + 50].unsqueeze(2).broadcast_to([128, 50, 36]))
            rq = rqp.tile([128, 36], F32, tag=f"rq{u}")
            nc.vector.tensor_reduce(
                out=rq, in_=wub.rearrange("p (t e) -> p e t", e=36),
                axis=mybir.AxisListType.X, op=mybir.AluOpType.add)
            mv_parts.setdefault(q, []).append(rq)
            if u == 3:
                r = mv_parts.pop(q)
                s01 = rqp.tile([128, 36], F32, tag="s01")
                nc.vector.tensor_add(s01, r[0], r[1])
                s23 = rqp.tile([128, 36], F32, tag="s23")
                nc.vector.tensor_add(s23, r[2], r[3])
                ot = outp.tile([128, 36], F32, tag="ot")
                nc.vector.tensor_add(ot, s01, s23)
                nc.gpsimd.dma_start(out=out_d[128 * q:128 * (q + 1), :], in_=ot)

        # score-store trigger tiles for each softmax window, then the mv
        # quarters of window q interleaved over the following tiles
        trig_t = {}
        for q in range(n_q):
            g = -(-q_rows * (q + 1) // g_rows)    # ceil
            trig_t[q] = g * GP - 1
        mv_sched = {}
        for q in range(n_q):
            for u in range(4):
                tt = trig_t[q] + 4 + 6 * u
                if tt < n_t:
                    mv_sched[tt] = (q, u)

        next_slab = 0
        next_q = 0
        pend = {}     # q -> [ubg, wt]
        mv_done = set()

        for t in range(n_t):
            c_lo, c_hi = 1024 * t, 1024 * t + 1024
            while next_slab < n_slab and slab_c * next_slab < c_hi + 4 * slab_c:
                emit_slab_load(next_slab)
                next_slab += 1
            if t % (n_t // n_q) == 2:
                q = t // (n_t // n_q)
                if q < n_q:
                    pend[q] = [emit_ubg_load(q), None]

            m1 = m1p.tile([80, 1024], F32, tag="m1")
            b0, b1_ = c_lo // T, (c_hi - 1) // T
            for b in range(b0, b1_ + 1):
                c0 = max(T * b, c_lo)
                c1 = min(T * b + T, c_hi)
                s = b // 32
                slot = s % NSLOT
                nc.tensor.matmul(
                    m1[0:80, c0 - c_lo:c1 - c_lo],
                    waug2[:, slot, 80 * (b % 32):80 * (b % 32) + 80],
                    ubq2[:, slot, c0 - slab_c * s:c1 - slab_c * s],
                    start=True, stop=True)

            h1t = h1p.tile([80, 1024], BF16, tag="h1")
            nc.scalar.activation(out=h1t, in_=m1, func=AF.Tanh, scale=0.5)

            m2 = m2p.tile([128, 512], F32, tag="m2")
            nc.tensor.matmul(m2[0:40, :], w2t, h1t[:, 0:512],
                             start=True, stop=True)
            nc.tensor.matmul(m2[64:104, :], w2t, h1t[:, 512:1024],
                             start=True, stop=True)

            h2t = h2p.tile([128, 512], BF16, tag="h2")
            nc.scalar.activation(out=h2t, in_=m2, func=AF.Tanh,
                                 bias=b2c, scale=0.5)

            # mm3: one matmul per h2 tile scores chunk-pair (2t, 2t+1) into
            # psum rows (2j, 2j+1) of group g = t // GP, j = t % GP
            j = t % GP
            if j == 0:
                m3 = m3p.tile([2 * GP, 512], F32, tag="m3")
            nc.tensor.matmul(
                m3, w3v[:, j, :], h2t[0:104, :],
                start=(j == 0), stop=(j == GP - 1))
            if j == GP - 1:
                g = t // GP
                sct = scp.tile([2 * GP, 512], BF16, tag="sct")
                nc.vector.tensor_scalar_add(sct, m3, b3c)
                nc.gpsimd.dma_start(
                    out=bass.AP(tensor=scb_d.tensor,
                                offset=scb_d.offset + g_rows * g,
                                ap=[[512, 2 * GP], [1, 512]]),
                    in_=sct)
                while next_q < n_q and g_rows * (g + 1) >= q_rows * (next_q + 1):
                    pend[next_q][1] = emit_softmax(next_q)
                    next_q += 1

            if t in mv_sched:
                q, u = mv_sched[t]
                emit_mv_quarter(q, u, pend[q][0], pend[q][1])
                mv_done.add((q, u))

        # tail: any quarters not interleaved above
        for q in range(n_q):
            for u in range(4):
                if (q, u) not in mv_done:
                    emit_mv_quarter(q, u, pend[q][0], pend[q][1])

    nc.compile()
    return nc


def host_prep(query_ad, user_behavior, user_behavior_length,
              W1, b1, W2, b2, W3, b3, bc):
    GP = 10
    q = np.asarray(query_ad, dtype=np.float32)
    ub = np.asarray(user_behavior, dtype=np.float32)
    lens = np.asarray(user_behavior_length)
    W1 = np.asarray(W1, dtype=np.float32)
    b1 = np.asarray(b1, dtype=np.float32)
    W2 = np.asarray(W2, dtype=np.float32)
    b2 = np.asarray(b2, dtype=np.float32)
    W3 = np.asarray(W3, dtype=np.float32)
    b3 = np.asarray(b3, dtype=np.float32)
    nb = q.shape[0]
    n_cores = nb // bc

    Wa, Wb, Wc, Wd = W1[0:36], W1[36:72], W1[72:108], W1[108:144]
    # per-batch folded mm1 weights: [nb, 37, 80]
    waug = np.empty((nb, 37, 80), dtype=np.float32)
    waug[:, 0:36, :] = (Wb - Wc)[None, :, :] + q[:, :, None] * Wd[None, :, :]
    waug[:, 36, :] = q @ (Wa + Wc) + b1[None, :]

    # sigmoid -> tanh folds (see module docstring)
    w2f = (0.5 * W2).astype(NPBF16)                        # [80,40]
    b2f = 0.5 * (b2 + 0.5 * W2.sum(axis=0))                # [40]
    w3f = 0.5 * W3[:, 0]                                   # [40]
    b3f = float(b3[0] + 0.5 * W3.sum())

    b2c = np.zeros((128, 1), dtype=np.float32)
    b2c[0:40, 0] = b2f
    b2c[64:104, 0] = b2f
    # mm3 lhsT variants: variant j ([104, 2*GP]) has w3 in column 2j
    # (h2 rows 0-39, even chunk of the pair) and column 2j+1 (rows 64-103)
    w3v = np.zeros((104, GP, 2 * GP), dtype=np.float32)
    for j in range(GP):
        w3v[0:40, j, 2 * j] = w3f
        w3v[64:104, j, 2 * j + 1] = w3f
    w3v = w3v.reshape(104, GP * 2 * GP).astype(NPBF16)
    b3c = np.full((2 * GP, 1), b3f, dtype=np.float32)
    iota = np.arange(T, dtype=np.float32).astype(NPBF16)

    ub16 = ub.astype(NPBF16)                               # [nb, T, 36]

    in_maps = []
    for c in range(n_cores):
        sl = slice(bc * c, bc * (c + 1))
        ubc = ub16[sl].reshape(bc * T, 36)
        ubt = np.empty((37, bc * T), dtype=NPBF16)
        ubt[0:36] = ubc.T
        ubt[36] = 1.0
        in_maps.append({
            "ubt": ubt,
            "ubn": np.ascontiguousarray(ubc),
            "waug": np.ascontiguousarray(
                waug[sl].transpose(1, 0, 2).reshape(37, bc * 80)
            ).astype(NPBF16),
            "w2t": w2f, "w3v": w3v, "b2c": b2c, "b3c": b3c,
            "lens2": np.ascontiguousarray(
                lens[sl].astype(np.float32).reshape(bc // 128, 128).T
            ).reshape(bc),
            "iota": iota,
        })
    return in_maps


_NC_CACHE = {}


def get_module(bc):
    if bc not in _NC_CACHE:
        _NC_CACHE[bc] = build_module(bc)
    return _NC_CACHE[bc]


def kernel(query_ad, user_behavior, user_behavior_length,
           W1, b1, W2, b2, W3, b3, trace=False):
    bc = B // N_CORES
    nc = get_module(bc)
    in_maps = host_prep(query_ad, user_behavior, user_behavior_length,
                        W1, b1, W2, b2, W3, b3, bc)
    res = run_bass_kernel_spmd(nc, in_maps, core_ids=list(range(N_CORES)),
                               trace=trace)
    outs = [res.results[c]["out"] for c in range(N_CORES)]
    full = np.concatenate(outs, axis=0).reshape(B, 1, 36)
    if trace:
        kernel.last_result = res
    return full


# revision 13
# speedup vs baseline: 3.7883x; 1.5478x over previous
"""DIN attention layer kernel for Trainium2 (8 NeuronCores, data-parallel batch).

Reference math per (b,t):
  x  = concat([q, ub, q-ub, q*ub], -1)             # [B,T,144]
  h1 = sigmoid(x @ W1 + b1)                        # [B,T,80]
  h2 = sigmoid(h1 @ W2 + b2)                       # [B,T,40]
  s  = h2 @ W3 + b3                                # [B,T,1]
  w  = softmax(s.T * mask)                         # [B,1,T]  (multiplicative mask)
  out = w @ ub                                     # [B,1,36]

Design (v2, bf16):
  * All matmuls in bf16 (1 cycle/column vs 4 for fp32); accumulation fp32.
  * Host pre-transposes ub -> ubT [37, rows] (with a ones row) so mm1's rhs
    loads are large contiguous DMAs; no on-chip transposes anywhere.
  * x@W1 fold: x@W1+b1 = ubT.T@((Wb-Wc)+diag(q_b)Wd) + (q_b(Wa+Wc)+b1);
    host bakes per-batch lhsT [37,80] (36 fold rows + 1 bias row).
  * sigmoid(x)=0.5+0.5*tanh(x/2): tanh+exp share one ACT table set; the
    0.5/0.5 affine folds into the next layer's weights/biases.
  * mm2 packs two 512-col chunks into PSUM partitions [0:40] and [64:104].
  * mm3 uses sparse-column lhsT variants so 20 consecutive 512-row chunks
    accumulate into contiguous psum partitions 0..19 (one matmul serves a
    chunk pair); scores copy out in [20,512] blocks and roundtrip DRAM to
    land batch-major [128,200] for softmax.
  * Weighted sum w@ub on DVE: natural-layout ub [128 batches, 200*36] times
    softmax weights broadcast along features, then a strided reduce over t.
"""

from contextlib import ExitStack

import numpy as np
import ml_dtypes

import concourse.bass as bass
import concourse.bacc as bacc
import concourse.tile as tile
from concourse import mybir
from concourse.bass_utils import run_bass_kernel_spmd

B, T, E = 4096, 200, 36
N_CORES = 8
F32 = mybir.dt.float32
BF16 = mybir.dt.bfloat16
NPBF16 = ml_dtypes.bfloat16
AF = mybir.ActivationFunctionType


def build_module(bc=512):
    rows = bc * T                  # 102400
    n_slab = bc // 32              # 16 slabs of 32 batches
    slab_c = 32 * T                # 6400 cols per slab
    n_t = rows // 1024             # 100 mm1 psum tiles (1024 cols each)
    n_q = bc // 128                # 4 softmax tiles of 128 batches
    q_rows = 128 * T               # 25600 rows per softmax tile
    GP = 10                        # h2 tiles (chunk pairs) per mm3 psum group
    g_rows = GP * 1024             # 10240 rows per mm3 group

    nc = bacc.Bacc(
        "TRN2", target_bir_lowering=False, debug=False,
        enable_asserts=False, num_devices=N_CORES,
    )

    ubt_d = nc.dram_tensor("ubt", [37, rows], BF16, kind="ExternalInput").ap()
    ub_d = nc.dram_tensor("ubn", [rows, 36], BF16, kind="ExternalInput").ap()
    waug_d = nc.dram_tensor("waug", [37, bc * 80], BF16, kind="ExternalInput").ap()
    w2_d = nc.dram_tensor("w2t", [80, 64], BF16, kind="ExternalInput").ap()
    w3_d = nc.dram_tensor("w3v", [104, GP * 2 * GP], BF16,
                          kind="ExternalInput").ap()
    b2_d = nc.dram_tensor("b2c", [128, 1], F32, kind="ExternalInput").ap()
    b3_d = nc.dram_tensor("b3c", [2 * GP, 1], F32, kind="ExternalInput").ap()
    lens_d = nc.dram_tensor("lens2", [bc], F32, kind="ExternalInput").ap()
    iota_d = nc.dram_tensor("iota", [T], BF16, kind="ExternalInput").ap()
    out_d = nc.dram_tensor("out", [bc, 36], F32, kind="ExternalOutput").ap()
    scb_d = nc.dram_tensor("scb", [rows], BF16, kind="Internal").ap()

    with tile.TileContext(nc) as tc, ExitStack() as es:
        cpool = es.enter_context(tc.tile_pool(name="consts", bufs=1))
        h1p = es.enter_context(tc.tile_pool(name="h1p", bufs=4))
        h2p = es.enter_context(tc.tile_pool(name="h2p", bufs=4))
        scp = es.enter_context(tc.tile_pool(name="scp", bufs=3))
        smp = es.enter_context(tc.tile_pool(name="smp", bufs=2))
        ubgp = es.enter_context(tc.tile_pool(name="ubgp", bufs=2))
        wubp = es.enter_context(tc.tile_pool(name="wubp", bufs=2))
        rqp = es.enter_context(tc.tile_pool(name="rqp", bufs=6))
        outp = es.enter_context(tc.tile_pool(name="outp", bufs=2))
        m1p = es.enter_context(tc.tile_pool(name="m1p", bufs=2, space="PSUM"))
        m2p = es.enter_context(tc.tile_pool(name="m2p", bufs=2, space="PSUM"))
        m3p = es.enter_context(tc.tile_pool(name="m3p", bufs=2, space="PSUM"))

        # ---- constants ----
        w2t = cpool.tile([80, 64], BF16)
        nc.sync.dma_start(out=w2t, in_=w2_d)
        w3v = cpool.tile([104, GP, 2 * GP], BF16)
        nc.sync.dma_start(out=w3v, in_=w3_d)
        b2c = cpool.tile([128, 1], F32)
        nc.sync.dma_start(out=b2c, in_=b2_d)
        b3c = cpool.tile([2 * GP, 1], F32)
        nc.sync.dma_start(out=b3c, in_=b3_d)
        lens_sb = cpool.tile([128, n_q], F32)
        nc.sync.dma_start(
            out=lens_sb,
            in_=bass.AP(tensor=lens_d.tensor, offset=lens_d.offset,
                        ap=[[n_q, 128], [1, n_q]]))
        iota_t = cpool.tile([128, T], BF16)
        nc.sync.dma_start(
            out=iota_t,
            in_=bass.AP(tensor=iota_d.tensor, offset=iota_d.offset,
                        ap=[[0, 128], [1, T]]))

        # manually rotated (6-deep) slab buffers
        NSLOT = 6
        ubq2 = cpool.tile([37, NSLOT, slab_c], BF16)  # rows 0-35 ubT, row 36 ones
        waug2 = cpool.tile([37, NSLOT, 32 * 80], BF16)  # per-batch folded W1

        def emit_slab_load(s):
            slot = s % NSLOT
            nc.gpsimd.dma_start(
                out=ubq2[:, slot, :],
                in_=bass.AP(tensor=ubt_d.tensor,
                            offset=ubt_d.offset + slab_c * s,
                            ap=[[rows, 37], [1, slab_c]]))
            nc.gpsimd.dma_start(
                out=waug2[:, slot, :],
                in_=bass.AP(tensor=waug_d.tensor,
                            offset=waug_d.offset + 32 * 80 * s,
                            ap=[[bc * 80, 37], [1, 32 * 80]]))

        def emit_ubg_load(q):
            ubg = ubgp.tile([128, T * 36], BF16, tag="ubg", name=f"ubg{q}")
            nc.gpsimd.dma_start(
                out=ubg,
                in_=bass.AP(tensor=ub_d.tensor,
                            offset=ub_d.offset + q_rows * 36 * q,
                            ap=[[T * 36, 128], [1, T * 36]]))
            return ubg

        def emit_softmax(q):
            sc_t = smp.tile([128, T], BF16, tag="sc", name=f"sc{q}")
            nc.sync.dma_start(
                out=sc_t,
                in_=bass.AP(tensor=scb_d.tensor,
                            offset=scb_d.offset + q_rows * q,
                            ap=[[T, 128], [1, T]]))
            mask = smp.tile([128, T], BF16, tag="mask")
            nc.vector.tensor_scalar(
                out=mask, in0=iota_t, scalar1=lens_sb[:, q:q + 1],
                scalar2=None, op0=mybir.AluOpType.is_lt)
            masked = smp.tile([128, T], BF16, tag="masked")
            nc.vector.tensor_mul(masked, sc_t, mask)
            negmax = smp.tile([128, 1], F32, tag="negmax")
            nc.vector.tensor_reduce(
                out=negmax, in_=masked, axis=mybir.AxisListType.X,
                op=mybir.AluOpType.max, negate=True)
            ew = smp.tile([128, T], BF16, tag="ew")
            sumexp = smp.tile([128, 1], F32, tag="sumexp")
            nc.scalar.activation(
                out=ew, in_=masked, func=AF.Exp,
                bias=negmax, accum_out=sumexp)
            rz = smp.tile([128, 1], F32, tag="rz")
            nc.vector.reciprocal(rz, sumexp)
            wt = smp.tile([128, T], BF16, tag="wt", name=f"wt{q}")
            nc.vector.tensor_scalar_mul(wt, ew, rz)
            return wt

        mv_parts = {}

        def emit_mv_quarter(q, u, ubg, wt):
            # wub = ub * w over a 50-t slice; reduce over t -> [128, 36]
            t0 = 50 * u
            wub = wubp.tile([128, 50 * 36], BF16, tag="wub")
            nc.vector.tensor_mul(
                wub.rearrange("p (t e) -> p t e", e=36),
                ubg[:, t0 * 36:(t0 + 50) * 36].rearrange(
                    "p (t e) -> p t e", e=36),
                wt[:, t0:t0 + 50].unsqueeze(2).broadcast_to([128, 50, 36]))
            rq = rqp.tile([128, 36], F32, tag=f"rq{u}")
            nc.vector.tensor_reduce(
                out=rq, in_=wub.rearrange("p (t e) -> p e t", e=36),
                axis=mybir.AxisListType.X, op=mybir.AluOpType.add)
            mv_parts.setdefault(q, []).append(rq)
            if u == 3:
                r = mv_parts.pop(q)
                s01 = rqp.tile([128, 36], F32, tag="s01")
                nc.vector.tensor_add(s01, r[0], r[1])
                s23 = rqp.tile([128, 36], F32, tag="s23")
                nc.vector.tensor_add(s23, r[2], r[3])
                ot = outp.tile([128, 36], F32, tag="ot")
                nc.vector.tensor_add(ot, s01, s23)
                nc.gpsimd.dma_start(out=out_d[128 * q:128 * (q + 1), :], in_=ot)

        # score-store trigger tiles for each softmax window, then the mv
        # quarters of window q interleaved over the following tiles
        trig_t = {}
        for q in range(n_q):
            g = -(-q_rows * (q + 1) // g_rows)    # ceil
            trig_t[q] = g * GP - 1
        mv_sched = {}
        for q in range(n_q):
            for u in range(4):
                tt = trig_t[q] + 4 + 6 * u
                if tt < n_t:
                    mv_sched[tt] = (q, u)

        next_slab = 0
        next_q = 0
        pend = {}     # q -> [ubg, wt]
        mv_done = set()

        for t in range(n_t):
            c_lo, c_hi = 1024 * t, 1024 * t + 1024
            while next_slab < n_slab and slab_c * next_slab < c_hi + 4 * slab_c:
                emit_slab_load(next_slab)
                next_slab += 1
            if t % (n_t // n_q) == 2:
                q = t // (n_t // n_q)
                if q < n_q:
                    pend[q] = [emit_ubg_load(q), None]

            m1 = m1p.tile([80, 1024], F32, tag="m1")
            b0, b1_ = c_lo // T, (c_hi - 1) // T
            for b in range(b0, b1_ + 1):
                s = b // 32
                slot = s % NSLOT
                lhsT = waug2[:, slot, 80 * (b % 32):80 * (b % 32) + 80]
                # split at psum bank boundaries (512 fp32 cols)
                c0 = max(T * b, c_lo)
                c_end = min(T * b + T, c_hi)
                while c0 < c_end:
                    c1 = min(c_end, c_lo + 512 * ((c0 - c_lo) // 512 + 1))
                    nc.tensor.matmul(
                        m1[0:80, c0 - c_lo:c1 - c_lo],
                        lhsT,
                        ubq2[:, slot, c0 - slab_c * s:c1 - slab_c * s],
                        start=True, stop=True)
                    c0 = c1

            h1t = h1p.tile([80, 1024], BF16, tag="h1")
            nc.scalar.activation(out=h1t, in_=m1, func=AF.Tanh, scale=0.5)

            m2 = m2p.tile([128, 512], F32, tag="m2")
            nc.tensor.matmul(m2[0:64, :], w2t, h1t[:, 0:512],
                             start=True, stop=True)
            nc.tensor.matmul(m2[64:128, :], w2t, h1t[:, 512:1024],
                             start=True, stop=True)

            h2t = h2p.tile([128, 512], BF16, tag="h2")
            nc.scalar.activation(out=h2t, in_=m2, func=AF.Tanh,
                                 bias=b2c, scale=0.5)

            # mm3: one matmul per h2 tile scores chunk-pair (2t, 2t+1) into
            # psum rows (2j, 2j+1) of group g = t // GP, j = t % GP
            j = t % GP
            if j == 0:
                m3 = m3p.tile([2 * GP, 512], F32, tag="m3")
            nc.tensor.matmul(
                m3, w3v[:, j, :], h2t[0:104, :],
                start=(j == 0), stop=(j == GP - 1))
            if j == GP - 1:
                g = t // GP
                sct = scp.tile([2 * GP, 512], BF16, tag="sct")
                nc.vector.tensor_scalar_add(sct, m3, b3c)
                nc.sync.dma_start(
                    out=bass.AP(tensor=scb_d.tensor,
                                offset=scb_d.offset + g_rows * g,
                                ap=[[512, 2 * GP], [1, 512]]),
                    in_=sct)
                while next_q < n_q and g_rows * (g + 1) >= q_rows * (next_q + 1):
                    pend[next_q][1] = emit_softmax(next_q)
                    next_q += 1

            if t in mv_sched:
                q, u = mv_sched[t]
                emit_mv_quarter(q, u, pend[q][0], pend[q][1])
                mv_done.add((q, u))

        # tail: any quarters not interleaved above
        for q in range(n_q):
            for u in range(4):
                if (q, u) not in mv_done:
                    emit_mv_quarter(q, u, pend[q][0], pend[q][1])

    nc.compile()
    return nc


def host_prep(query_ad, user_behavior, user_behavior_length,
              W1, b1, W2, b2, W3, b3, bc):
    GP = 10
    q = np.asarray(query_ad, dtype=np.float32)
    ub = np.asarray(user_behavior, dtype=np.float32)
    lens = np.asarray(user_behavior_length)
    W1 = np.asarray(W1, dtype=np.float32)
    b1 = np.asarray(b1, dtype=np.float32)
    W2 = np.asarray(W2, dtype=np.float32)
    b2 = np.asarray(b2, dtype=np.float32)
    W3 = np.asarray(W3, dtype=np.float32)
    b3 = np.asarray(b3, dtype=np.float32)
    nb = q.shape[0]
    n_cores = nb // bc

    Wa, Wb, Wc, Wd = W1[0:36], W1[36:72], W1[72:108], W1[108:144]
    # per-batch folded mm1 weights: [nb, 37, 80]
    waug = np.empty((nb, 37, 80), dtype=np.float32)
    waug[:, 0:36, :] = (Wb - Wc)[None, :, :] + q[:, :, None] * Wd[None, :, :]
    waug[:, 36, :] = q @ (Wa + Wc) + b1[None, :]

    # sigmoid -> tanh folds (see module docstring)
    w2f = np.zeros((80, 64), dtype=np.float32)             # [80,64] zero-padded
    w2f[:, 0:40] = 0.5 * W2
    w2f = w2f.astype(NPBF16)
    b2f = 0.5 * (b2 + 0.5 * W2.sum(axis=0))                # [40]
    w3f = 0.5 * W3[:, 0]                                   # [40]
    b3f = float(b3[0] + 0.5 * W3.sum())

    b2c = np.zeros((128, 1), dtype=np.float32)
    b2c[0:40, 0] = b2f
    b2c[64:104, 0] = b2f
    # mm3 lhsT variants: variant j ([104, 2*GP]) has w3 in column 2j
    # (h2 rows 0-39, even chunk of the pair) and column 2j+1 (rows 64-103)
    w3v = np.zeros((104, GP, 2 * GP), dtype=np.float32)
    for j in range(GP):
        w3v[0:40, j, 2 * j] = w3f
        w3v[64:104, j, 2 * j + 1] = w3f
    w3v = w3v.reshape(104, GP * 2 * GP).astype(NPBF16)
    b3c = np.full((2 * GP, 1), b3f, dtype=np.float32)
    iota = np.arange(T, dtype=np.float32).astype(NPBF16)

    ub16 = ub.astype(NPBF16)                               # [nb, T, 36]

    in_maps = []
    for c in range(n_cores):
        sl = slice(bc * c, bc * (c + 1))
        ubc = ub16[sl].reshape(bc * T, 36)
        ubt = np.empty((37, bc * T), dtype=NPBF16)
        ubt[0:36] = ubc.T
        ubt[36] = 1.0
        in_maps.append({
            "ubt": ubt,
            "ubn": np.ascontiguousarray(ubc),
            "waug": np.ascontiguousarray(
                waug[sl].transpose(1, 0, 2).reshape(37, bc * 80)
            ).astype(NPBF16),
            "w2t": w2f, "w3v": w3v, "b2c": b2c, "b3c": b3c,
            "lens2": np.ascontiguousarray(
                lens[sl].astype(np.float32).reshape(bc // 128, 128).T
            ).reshape(bc),
            "iota": iota,
        })
    return in_maps


_NC_CACHE = {}


def get_module(bc):
    if bc not in _NC_CACHE:
        _NC_CACHE[bc] = build_module(bc)
    return _NC_CACHE[bc]


def kernel(query_ad, user_behavior, user_behavior_length,
           W1, b1, W2, b2, W3, b3, trace=False):
    bc = B // N_CORES
    nc = get_module(bc)
    in_maps = host_prep(query_ad, user_behavior, user_behavior_length,
                        W1, b1, W2, b2, W3, b3, bc)
    res = run_bass_kernel_spmd(nc, in_maps, core_ids=list(range(N_CORES)),
                               trace=trace)
    outs = [res.results[c]["out"] for c in range(N_CORES)]
    full = np.concatenate(outs, axis=0).reshape(B, 1, 36)
    if trace:
        kernel.last_result = res
    return full


# revision 15
# speedup vs baseline: 3.8540x; 1.0173x over previous
"""DIN attention layer kernel for Trainium2 (8 NeuronCores, data-parallel batch).

Reference math per (b,t):
  x  = concat([q, ub, q-ub, q*ub], -1)             # [B,T,144]
  h1 = sigmoid(x @ W1 + b1)                        # [B,T,80]
  h2 = sigmoid(h1 @ W2 + b2)                       # [B,T,40]
  s  = h2 @ W3 + b3                                # [B,T,1]
  w  = softmax(s.T * mask)                         # [B,1,T]  (multiplicative mask)
  out = w @ ub                                     # [B,1,36]

Design (v2, bf16):
  * All matmuls in bf16 (1 cycle/column vs 4 for fp32); accumulation fp32.
  * Host pre-transposes ub -> ubT [37, rows] (with a ones row) so mm1's rhs
    loads are large contiguous DMAs; no on-chip transposes anywhere.
  * x@W1 fold: x@W1+b1 = ubT.T@((Wb-Wc)+diag(q_b)Wd) + (q_b(Wa+Wc)+b1);
    host bakes per-batch lhsT [37,80] (36 fold rows + 1 bias row).
  * sigmoid(x)=0.5+0.5*tanh(x/2): tanh+exp share one ACT table set; the
    0.5/0.5 affine folds into the next layer's weights/biases.
  * mm2 packs two 512-col chunks into PSUM partitions [0:40] and [64:104].
  * mm3 uses sparse-column lhsT variants so 20 consecutive 512-row chunks
    accumulate into contiguous psum partitions 0..19 (one matmul serves a
    chunk pair); scores copy out in [20,512] blocks and roundtrip DRAM to
    land batch-major [128,200] for softmax.
  * Weighted sum w@ub on DVE: natural-layout ub [128 batches, 200*36] times
    softmax weights broadcast along features, then a strided reduce over t.
"""

from contextlib import ExitStack

import numpy as np
import ml_dtypes

import concourse.bass as bass
import concourse.bacc as bacc
import concourse.tile as tile
from concourse import mybir
from concourse.bass_utils import run_bass_kernel_spmd

B, T, E = 4096, 200, 36
N_CORES = 8
F32 = mybir.dt.float32
BF16 = mybir.dt.bfloat16
NPBF16 = ml_dtypes.bfloat16
AF = mybir.ActivationFunctionType


def build_module(bc=512):
    rows = bc * T                  # 102400
    n_slab = bc // 32              # 16 slabs of 32 batches
    slab_c = 32 * T                # 6400 cols per slab
    n_t = rows // 1024             # 100 mm1 psum tiles (1024 cols each)
    n_q = bc // 128                # 4 softmax tiles of 128 batches
    q_rows = 128 * T               # 25600 rows per softmax tile
    GP = 5                         # h2 tiles (chunk pairs) per mm3 psum group
    g_rows = GP * 1024             # 10240 rows per mm3 group

    nc = bacc.Bacc(
        "TRN2", target_bir_lowering=False, debug=False,
        enable_asserts=False, num_devices=N_CORES,
    )

    ubt_d = nc.dram_tensor("ubt", [37, rows], BF16, kind="ExternalInput").ap()
    ub_d = nc.dram_tensor("ubn", [rows, 36], BF16, kind="ExternalInput").ap()
    waug_d = nc.dram_tensor("waug", [37, bc * 80], BF16, kind="ExternalInput").ap()
    w2_d = nc.dram_tensor("w2t", [80, 64], BF16, kind="ExternalInput").ap()
    w3_d = nc.dram_tensor("w3v", [104, GP * 2 * GP], BF16,
                          kind="ExternalInput").ap()
    b2_d = nc.dram_tensor("b2c", [128, 1], F32, kind="ExternalInput").ap()
    b3_d = nc.dram_tensor("b3c", [2 * GP, 1], F32, kind="ExternalInput").ap()
    lens_d = nc.dram_tensor("lens2", [bc], F32, kind="ExternalInput").ap()
    iota_d = nc.dram_tensor("iota", [T], BF16, kind="ExternalInput").ap()
    out_d = nc.dram_tensor("out", [bc, 36], F32, kind="ExternalOutput").ap()
    scb_d = nc.dram_tensor("scb", [rows], BF16, kind="Internal").ap()

    with tile.TileContext(nc) as tc, ExitStack() as es:
        cpool = es.enter_context(tc.tile_pool(name="consts", bufs=1))
        h1p = es.enter_context(tc.tile_pool(name="h1p", bufs=4))
        h2p = es.enter_context(tc.tile_pool(name="h2p", bufs=4))
        scp = es.enter_context(tc.tile_pool(name="scp", bufs=3))
        smp = es.enter_context(tc.tile_pool(name="smp", bufs=2))
        ubgp = es.enter_context(tc.tile_pool(name="ubgp", bufs=2))
        wubp = es.enter_context(tc.tile_pool(name="wubp", bufs=2))
        rqp = es.enter_context(tc.tile_pool(name="rqp", bufs=6))
        outp = es.enter_context(tc.tile_pool(name="outp", bufs=2))
        m1p = es.enter_context(tc.tile_pool(name="m1p", bufs=2, space="PSUM"))
        m2p = es.enter_context(tc.tile_pool(name="m2p", bufs=2, space="PSUM"))
        m3p = es.enter_context(tc.tile_pool(name="m3p", bufs=2, space="PSUM"))

        # ---- constants ----
        w2t = cpool.tile([80, 64], BF16)
        nc.sync.dma_start(out=w2t, in_=w2_d)
        w3v = cpool.tile([104, GP, 2 * GP], BF16)
        nc.sync.dma_start(out=w3v, in_=w3_d)
        b2c = cpool.tile([128, 1], F32)
        nc.sync.dma_start(out=b2c, in_=b2_d)
        b3c = cpool.tile([2 * GP, 1], F32)
        nc.sync.dma_start(out=b3c, in_=b3_d)
        lens_sb = cpool.tile([128, n_q], F32)
        nc.sync.dma_start(
            out=lens_sb,
            in_=bass.AP(tensor=lens_d.tensor, offset=lens_d.offset,
                        ap=[[n_q, 128], [1, n_q]]))
        iota_t = cpool.tile([128, T], BF16)
        nc.sync.dma_start(
            out=iota_t,
            in_=bass.AP(tensor=iota_d.tensor, offset=iota_d.offset,
                        ap=[[0, 128], [1, T]]))

        # manually rotated (6-deep) slab buffers
        NSLOT = 6
        ubq2 = cpool.tile([37, NSLOT, slab_c], BF16)  # rows 0-35 ubT, row 36 ones
        waug2 = cpool.tile([37, NSLOT, 32 * 80], BF16)  # per-batch folded W1

        def emit_slab_load(s):
            slot = s % NSLOT
            nc.gpsimd.dma_start(
                out=ubq2[:, slot, :],
                in_=bass.AP(tensor=ubt_d.tensor,
                            offset=ubt_d.offset + slab_c * s,
                            ap=[[rows, 37], [1, slab_c]]))
            nc.gpsimd.dma_start(
                out=waug2[:, slot, :],
                in_=bass.AP(tensor=waug_d.tensor,
                            offset=waug_d.offset + 32 * 80 * s,
                            ap=[[bc * 80, 37], [1, 32 * 80]]))

        def emit_ubg_load(q):
            ubg = ubgp.tile([128, T * 36], BF16, tag="ubg", name=f"ubg{q}")
            nc.gpsimd.dma_start(
                out=ubg,
                in_=bass.AP(tensor=ub_d.tensor,
                            offset=ub_d.offset + q_rows * 36 * q,
                            ap=[[T * 36, 128], [1, T * 36]]))
            return ubg

        def emit_softmax(q):
            sc_t = smp.tile([128, T], BF16, tag="sc", name=f"sc{q}")
            nc.sync.dma_start(
                out=sc_t,
                in_=bass.AP(tensor=scb_d.tensor,
                            offset=scb_d.offset + q_rows * q,
                            ap=[[T, 128], [1, T]]))
            mask = smp.tile([128, T], BF16, tag="mask")
            nc.vector.tensor_scalar(
                out=mask, in0=iota_t, scalar1=lens_sb[:, q:q + 1],
                scalar2=None, op0=mybir.AluOpType.is_lt)
            masked = smp.tile([128, T], BF16, tag="masked")
            nc.vector.tensor_mul(masked, sc_t, mask)
            negmax = smp.tile([128, 1], F32, tag="negmax")
            nc.vector.tensor_reduce(
                out=negmax, in_=masked, axis=mybir.AxisListType.X,
                op=mybir.AluOpType.max, negate=True)
            ew = smp.tile([128, T], BF16, tag="ew")
            sumexp = smp.tile([128, 1], F32, tag="sumexp")
            nc.scalar.activation(
                out=ew, in_=masked, func=AF.Exp,
                bias=negmax, accum_out=sumexp)
            rz = smp.tile([128, 1], F32, tag="rz")
            nc.vector.reciprocal(rz, sumexp)
            wt = smp.tile([128, T], BF16, tag="wt", name=f"wt{q}")
            nc.vector.tensor_scalar_mul(wt, ew, rz)
            return wt

        mv_parts = {}

        def emit_mv_quarter(q, u, ubg, wt):
            # u=0/1: wub halves = ub * w (bf16, 2x mode); u=2: fold the t
            # halves together; u=3: strided add-reduce over t -> [128, 36]
            if u in (0, 1):
                t0 = 100 * u
                wub = wubp.tile([128, 100 * 36], BF16, tag=f"wub{u}",
                                name=f"wub{u}_{q}")
                nc.vector.tensor_mul(
                    wub.rearrange("p (t e) -> p t e", e=36),
                    ubg[:, t0 * 36:(t0 + 100) * 36].rearrange(
                        "p (t e) -> p t e", e=36),
                    wt[:, t0:t0 + 100].unsqueeze(2).broadcast_to(
                        [128, 100, 36]))
                mv_parts.setdefault(q, []).append(wub)
            elif u == 2:
                a, b = mv_parts.pop(q)
                w100 = wubp.tile([128, 100 * 36], BF16, tag="w100",
                                 name=f"w100_{q}")
                nc.vector.tensor_add(w100, a, b)
                mv_parts[q] = w100
            else:
                w100 = mv_parts.pop(q)
                ot = outp.tile([128, 36], F32, tag="ot")
                nc.vector.tensor_reduce(
                    out=ot, in_=w100.rearrange("p (t e) -> p e t", e=36),
                    axis=mybir.AxisListType.X, op=mybir.AluOpType.add)
                nc.gpsimd.dma_start(out=out_d[128 * q:128 * (q + 1), :], in_=ot)

        # score-store trigger tiles for each softmax window, then the mv
        # quarters of window q interleaved over the following tiles
        trig_t = {}
        for q in range(n_q):
            g = -(-q_rows * (q + 1) // g_rows)    # ceil
            trig_t[q] = g * GP - 1
        mv_sched = {}
        for q in range(n_q):
            for u in range(4):
                tt = trig_t[q] + 4 + 6 * u
                if tt < n_t:
                    mv_sched[tt] = (q, u)

        next_slab = 0
        next_q = 0
        pend = {}     # q -> [ubg, wt]
        mv_done = set()

        for t in range(n_t):
            c_lo, c_hi = 1024 * t, 1024 * t + 1024
            while next_slab < n_slab and slab_c * next_slab < c_hi + 4 * slab_c:
                emit_slab_load(next_slab)
                next_slab += 1
            if t % (n_t // n_q) == 2:
                q = t // (n_t // n_q)
                if q < n_q:
                    pend[q] = [emit_ubg_load(q), None]

            m1 = m1p.tile([80, 1024], F32, tag="m1")
            b0, b1_ = c_lo // T, (c_hi - 1) // T
            for b in range(b0, b1_ + 1):
                s = b // 32
                slot = s % NSLOT
                lhsT = waug2[:, slot, 80 * (b % 32):80 * (b % 32) + 80]
                # split at psum bank boundaries (512 fp32 cols)
                c0 = max(T * b, c_lo)
                c_end = min(T * b + T, c_hi)
                while c0 < c_end:
                    c1 = min(c_end, c_lo + 512 * ((c0 - c_lo) // 512 + 1))
                    nc.tensor.matmul(
                        m1[0:80, c0 - c_lo:c1 - c_lo],
                        lhsT,
                        ubq2[:, slot, c0 - slab_c * s:c1 - slab_c * s],
                        start=True, stop=True)
                    c0 = c1

            h1t = h1p.tile([80, 1024], BF16, tag="h1")
            nc.scalar.activation(out=h1t, in_=m1, func=AF.Tanh, scale=0.5)

            m2 = m2p.tile([128, 512], F32, tag="m2")
            nc.tensor.matmul(m2[0:64, :], w2t, h1t[:, 0:512],
                             start=True, stop=True)
            nc.tensor.matmul(m2[64:128, :], w2t, h1t[:, 512:1024],
                             start=True, stop=True)

            h2t = h2p.tile([128, 512], BF16, tag="h2")
            nc.scalar.activation(out=h2t, in_=m2, func=AF.Tanh,
                                 bias=b2c, scale=0.5)

            # mm3: one matmul per h2 tile scores chunk-pair (2t, 2t+1) into
            # psum rows (2j, 2j+1) of group g = t // GP, j = t % GP
            j = t % GP
            if j == 0:
                m3 = m3p.tile([2 * GP, 512], F32, tag="m3")
            nc.tensor.matmul(
                m3, w3v[:, j, :], h2t[0:104, :],
                start=(j == 0), stop=(j == GP - 1))
            if j == GP - 1:
                g = t // GP
                sct = scp.tile([2 * GP, 512], BF16, tag="sct")
                nc.vector.tensor_scalar_add(sct, m3, b3c)
                nc.sync.dma_start(
                    out=bass.AP(tensor=scb_d.tensor,
                                offset=scb_d.offset + g_rows * g,
                                ap=[[512, 2 * GP], [1, 512]]),
                    in_=sct)
                while next_q < n_q and g_rows * (g + 1) >= q_rows * (next_q + 1):
                    pend[next_q][1] = emit_softmax(next_q)
                    next_q += 1

            if t in mv_sched:
                q, u = mv_sched[t]
                emit_mv_quarter(q, u, pend[q][0], pend[q][1])
                mv_done.add((q, u))

        # tail: any quarters not interleaved above
        for q in range(n_q):
            for u in range(4):
                if (q, u) not in mv_done:
                    emit_mv_quarter(q, u, pend[q][0], pend[q][1])

    nc.compile()
    return nc


def host_prep(query_ad, user_behavior, user_behavior_length,
              W1, b1, W2, b2, W3, b3, bc):
    GP = 5
    q = np.asarray(query_ad, dtype=np.float32)
    ub = np.asarray(user_behavior, dtype=np.float32)
    lens = np.asarray(user_behavior_length)
    W1 = np.asarray(W1, dtype=np.float32)
    b1 = np.asarray(b1, dtype=np.float32)
    W2 = np.asarray(W2, dtype=np.float32)
    b2 = np.asarray(b2, dtype=np.float32)
    W3 = np.asarray(W3, dtype=np.float32)
    b3 = np.asarray(b3, dtype=np.float32)
    nb = q.shape[0]
    n_cores = nb // bc

    Wa, Wb, Wc, Wd = W1[0:36], W1[36:72], W1[72:108], W1[108:144]
    # per-batch folded mm1 weights: [nb, 37, 80]
    waug = np.empty((nb, 37, 80), dtype=np.float32)
    waug[:, 0:36, :] = (Wb - Wc)[None, :, :] + q[:, :, None] * Wd[None, :, :]
    waug[:, 36, :] = q @ (Wa + Wc) + b1[None, :]

    # sigmoid -> tanh folds (see module docstring)
    w2f = np.zeros((80, 64), dtype=np.float32)             # [80,64] zero-padded
    w2f[:, 0:40] = 0.5 * W2
    w2f = w2f.astype(NPBF16)
    b2f = 0.5 * (b2 + 0.5 * W2.sum(axis=0))                # [40]
    w3f = 0.5 * W3[:, 0]                                   # [40]
    b3f = float(b3[0] + 0.5 * W3.sum())

    b2c = np.zeros((128, 1), dtype=np.float32)
    b2c[0:40, 0] = b2f
    b2c[64:104, 0] = b2f
    # mm3 lhsT variants: variant j ([104, 2*GP]) has w3 in column 2j
    # (h2 rows 0-39, even chunk of the pair) and column 2j+1 (rows 64-103)
    w3v = np.zeros((104, GP, 2 * GP), dtype=np.float32)
    for j in range(GP):
        w3v[0:40, j, 2 * j] = w3f
        w3v[64:104, j, 2 * j + 1] = w3f
    w3v = w3v.reshape(104, GP * 2 * GP).astype(NPBF16)
    b3c = np.full((2 * GP, 1), b3f, dtype=np.float32)
    iota = np.arange(T, dtype=np.float32).astype(NPBF16)

    ub16 = ub.astype(NPBF16)                               # [nb, T, 36]

    in_maps = []
    for c in range(n_cores):
        sl = slice(bc * c, bc * (c + 1))
        ubc = ub16[sl].reshape(bc * T, 36)
        ubt = np.empty((37, bc * T), dtype=NPBF16)
        ubt[0:36] = ubc.T
        ubt[36] = 1.0
        in_maps.append({
            "ubt": ubt,
            "ubn": np.ascontiguousarray(ubc),
            "waug": np.ascontiguousarray(
                waug[sl].transpose(1, 0, 2).reshape(37, bc * 80)
            ).astype(NPBF16),
            "w2t": w2f, "w3v": w3v, "b2c": b2c, "b3c": b3c,
            "lens2": np.ascontiguousarray(
                lens[sl].astype(np.float32).reshape(bc // 128, 128).T
            ).reshape(bc),
            "iota": iota,
        })
    return in_maps


_NC_CACHE = {}


def get_module(bc):
    if bc not in _NC_CACHE:
        _NC_CACHE[bc] = build_module(bc)
    return _NC_CACHE[bc]


def kernel(query_ad, user_behavior, user_behavior_length,
           W1, b1, W2, b2, W3, b3, trace=False):
    bc = B // N_CORES
    nc = get_module(bc)
    in_maps = host_prep(query_ad, user_behavior, user_behavior_length,
                        W1, b1, W2, b2, W3, b3, bc)
    res = run_bass_kernel_spmd(nc, in_maps, core_ids=list(range(N_CORES)),
                               trace=trace)
    outs = [res.results[c]["out"] for c in range(N_CORES)]
    full = np.concatenate(outs, axis=0).reshape(B, 1, 36)
    if trace:
        kernel.last_result = res
    return full


# revision 16
# speedup vs baseline: 4.0587x; 1.0531x over previous
"""DIN attention layer kernel for Trainium2 (8 NeuronCores, data-parallel batch).

Reference math per (b,t):
  x  = concat([q, ub, q-ub, q*ub], -1)             # [B,T,144]
  h1 = sigmoid(x @ W1 + b1)                        # [B,T,80]
  h2 = sigmoid(h1 @ W2 + b2)                       # [B,T,40]
  s  = h2 @ W3 + b3                                # [B,T,1]
  w  = softmax(s.T * mask)                         # [B,1,T]  (multiplicative mask)
  out = w @ ub                                     # [B,1,36]

Design (v2, bf16):
  * All matmuls in bf16 (1 cycle/column vs 4 for fp32); accumulation fp32.
  * Host pre-transposes ub -> ubT [37, rows] (with a ones row) so mm1's rhs
    loads are large contiguous DMAs; no on-chip transposes anywhere.
  * x@W1 fold: x@W1+b1 = ubT.T@((Wb-Wc)+diag(q_b)Wd) + (q_b(Wa+Wc)+b1);
    host bakes per-batch lhsT [37,80] (36 fold rows + 1 bias row).
  * sigmoid(x)=0.5+0.5*tanh(x/2): tanh+exp share one ACT table set; the
    0.5/0.5 affine folds into the next layer's weights/biases.
  * mm2 packs two 512-col chunks into PSUM partitions [0:40] and [64:104].
  * mm3 uses sparse-column lhsT variants so 20 consecutive 512-row chunks
    accumulate into contiguous psum partitions 0..19 (one matmul serves a
    chunk pair); scores copy out in [20,512] blocks and roundtrip DRAM to
    land batch-major [128,200] for softmax.
  * Weighted sum w@ub on DVE: natural-layout ub [128 batches, 200*36] times
    softmax weights broadcast along features, then a strided reduce over t.
"""

from contextlib import ExitStack

import numpy as np
import ml_dtypes

import concourse.bass as bass
import concourse.bacc as bacc
import concourse.tile as tile
from concourse import mybir
from concourse.bass_utils import run_bass_kernel_spmd

B, T, E = 4096, 200, 36
N_CORES = 8
F32 = mybir.dt.float32
BF16 = mybir.dt.bfloat16
NPBF16 = ml_dtypes.bfloat16
AF = mybir.ActivationFunctionType


def build_module(bc=512):
    rows = bc * T                  # 102400
    n_slab = bc // 32              # 16 slabs of 32 batches
    slab_c = 32 * T                # 6400 cols per slab
    n_t = rows // 1024             # 100 mm1 psum tiles (1024 cols each)
    n_q = bc // 128                # 4 softmax tiles of 128 batches
    q_rows = 128 * T               # 25600 rows per softmax tile
    GP = 5                         # h2 tiles (chunk pairs) per mm3 psum group
    g_rows = GP * 1024             # 10240 rows per mm3 group

    nc = bacc.Bacc(
        "TRN2", target_bir_lowering=False, debug=False,
        enable_asserts=False, num_devices=N_CORES,
    )

    ubqt_d = nc.dram_tensor("ubqt", [72, rows], BF16, kind="ExternalInput").ap()
    ub_d = nc.dram_tensor("ubn", [rows, 36], BF16, kind="ExternalInput").ap()
    s32_d = nc.dram_tensor("s32c", [32, slab_c], BF16, kind="ExternalInput").ap()
    w1s_d = nc.dram_tensor("w1s", [104, n_slab * 80], BF16,
                           kind="ExternalInput").ap()
    w2_d = nc.dram_tensor("w2t", [80, 64], BF16, kind="ExternalInput").ap()
    w3_d = nc.dram_tensor("w3v", [104, GP * 2 * GP], BF16,
                          kind="ExternalInput").ap()
    b2_d = nc.dram_tensor("b2c", [128, 1], F32, kind="ExternalInput").ap()
    b3_d = nc.dram_tensor("b3c", [2 * GP, 1], F32, kind="ExternalInput").ap()
    lens_d = nc.dram_tensor("lens2", [bc], F32, kind="ExternalInput").ap()
    iota_d = nc.dram_tensor("iota", [T], BF16, kind="ExternalInput").ap()
    out_d = nc.dram_tensor("out", [bc, 36], F32, kind="ExternalOutput").ap()
    scb_d = nc.dram_tensor("scb", [rows], BF16, kind="Internal").ap()

    with tile.TileContext(nc) as tc, ExitStack() as es:
        cpool = es.enter_context(tc.tile_pool(name="consts", bufs=1))
        h1p = es.enter_context(tc.tile_pool(name="h1p", bufs=4))
        h2p = es.enter_context(tc.tile_pool(name="h2p", bufs=4))
        scp = es.enter_context(tc.tile_pool(name="scp", bufs=3))
        smp = es.enter_context(tc.tile_pool(name="smp", bufs=2))
        ubgp = es.enter_context(tc.tile_pool(name="ubgp", bufs=2))
        wubp = es.enter_context(tc.tile_pool(name="wubp", bufs=2))
        rqp = es.enter_context(tc.tile_pool(name="rqp", bufs=6))
        outp = es.enter_context(tc.tile_pool(name="outp", bufs=2))
        m1p = es.enter_context(tc.tile_pool(name="m1p", bufs=2, space="PSUM"))
        m2p = es.enter_context(tc.tile_pool(name="m2p", bufs=2, space="PSUM"))
        m3p = es.enter_context(tc.tile_pool(name="m3p", bufs=2, space="PSUM"))

        # ---- constants ----
        w2t = cpool.tile([80, 64], BF16)
        nc.sync.dma_start(out=w2t, in_=w2_d)
        w3v = cpool.tile([104, GP, 2 * GP], BF16)
        nc.sync.dma_start(out=w3v, in_=w3_d)
        b2c = cpool.tile([128, 1], F32)
        nc.sync.dma_start(out=b2c, in_=b2_d)
        b3c = cpool.tile([2 * GP, 1], F32)
        nc.sync.dma_start(out=b3c, in_=b3_d)
        lens_sb = cpool.tile([128, n_q], F32)
        nc.sync.dma_start(
            out=lens_sb,
            in_=bass.AP(tensor=lens_d.tensor, offset=lens_d.offset,
                        ap=[[n_q, 128], [1, n_q]]))
        iota_t = cpool.tile([128, T], BF16)
        nc.sync.dma_start(
            out=iota_t,
            in_=bass.AP(tensor=iota_d.tensor, offset=iota_d.offset,
                        ap=[[0, 128], [1, T]]))

        # manually rotated (6-deep) slab buffers. mm1 rhs layout per slab:
        # rows 0-31 = batch-selector S32 (constant), 32-67 = ubT, 68-103 = qubT
        NSLOT = 6
        ubqt3 = cpool.tile([104, NSLOT, slab_c], BF16)
        w1s2 = cpool.tile([104, NSLOT, 80], BF16)
        for _slot in range(NSLOT):
            nc.sync.dma_start(out=ubqt3[0:32, _slot, :], in_=s32_d)

        def emit_slab_load(s):
            slot = s % NSLOT
            nc.gpsimd.dma_start(
                out=ubqt3[32:104, slot, :],
                in_=bass.AP(tensor=ubqt_d.tensor,
                            offset=ubqt_d.offset + slab_c * s,
                            ap=[[rows, 72], [1, slab_c]]))
            nc.gpsimd.dma_start(
                out=w1s2[:, slot, :],
                in_=bass.AP(tensor=w1s_d.tensor,
                            offset=w1s_d.offset + 80 * s,
                            ap=[[n_slab * 80, 104], [1, 80]]))

        def emit_ubg_load(q):
            ubg = ubgp.tile([128, T * 36], BF16, tag="ubg", name=f"ubg{q}")
            nc.gpsimd.dma_start(
                out=ubg,
                in_=bass.AP(tensor=ub_d.tensor,
                            offset=ub_d.offset + q_rows * 36 * q,
                            ap=[[T * 36, 128], [1, T * 36]]))
            return ubg

        def emit_softmax(q):
            sc_t = smp.tile([128, T], BF16, tag="sc", name=f"sc{q}")
            nc.sync.dma_start(
                out=sc_t,
                in_=bass.AP(tensor=scb_d.tensor,
                            offset=scb_d.offset + q_rows * q,
                            ap=[[T, 128], [1, T]]))
            mask = smp.tile([128, T], BF16, tag="mask")
            nc.vector.tensor_scalar(
                out=mask, in0=iota_t, scalar1=lens_sb[:, q:q + 1],
                scalar2=None, op0=mybir.AluOpType.is_lt)
            masked = smp.tile([128, T], BF16, tag="masked")
            nc.vector.tensor_mul(masked, sc_t, mask)
            negmax = smp.tile([128, 1], F32, tag="negmax")
            nc.vector.tensor_reduce(
                out=negmax, in_=masked, axis=mybir.AxisListType.X,
                op=mybir.AluOpType.max, negate=True)
            ew = smp.tile([128, T], BF16, tag="ew")
            sumexp = smp.tile([128, 1], F32, tag="sumexp")
            nc.scalar.activation(
                out=ew, in_=masked, func=AF.Exp,
                bias=negmax, accum_out=sumexp)
            rz = smp.tile([128, 1], F32, tag="rz")
            nc.vector.reciprocal(rz, sumexp)
            wt = smp.tile([128, T], BF16, tag="wt", name=f"wt{q}")
            nc.vector.tensor_scalar_mul(wt, ew, rz)
            return wt

        mv_parts = {}

        def emit_mv_quarter(q, u, ubg, wt):
            # u=0/1: wub halves = ub * w (bf16, 2x mode); u=2: fold the t
            # halves together; u=3: strided add-reduce over t -> [128, 36]
            if u in (0, 1):
                t0 = 100 * u
                wub = wubp.tile([128, 100 * 36], BF16, tag=f"wub{u}",
                                name=f"wub{u}_{q}")
                nc.vector.tensor_mul(
                    wub.rearrange("p (t e) -> p t e", e=36),
                    ubg[:, t0 * 36:(t0 + 100) * 36].rearrange(
                        "p (t e) -> p t e", e=36),
                    wt[:, t0:t0 + 100].unsqueeze(2).broadcast_to(
                        [128, 100, 36]))
                mv_parts.setdefault(q, []).append(wub)
            elif u == 2:
                a, b = mv_parts.pop(q)
                w100 = wubp.tile([128, 100 * 36], BF16, tag="w100",
                                 name=f"w100_{q}")
                nc.vector.tensor_add(w100, a, b)
                mv_parts[q] = w100
            else:
                w100 = mv_parts.pop(q)
                ot = outp.tile([128, 36], F32, tag="ot")
                nc.vector.tensor_reduce(
                    out=ot, in_=w100.rearrange("p (t e) -> p e t", e=36),
                    axis=mybir.AxisListType.X, op=mybir.AluOpType.add)
                nc.gpsimd.dma_start(out=out_d[128 * q:128 * (q + 1), :], in_=ot)

        # score-store trigger tiles for each softmax window, then the mv
        # quarters of window q interleaved over the following tiles
        trig_t = {}
        for q in range(n_q):
            g = -(-q_rows * (q + 1) // g_rows)    # ceil
            trig_t[q] = g * GP - 1
        mv_sched = {}
        for q in range(n_q):
            for u in range(4):
                tt = trig_t[q] + 4 + 6 * u
                if tt < n_t:
                    mv_sched[tt] = (q, u)

        next_slab = 0
        next_q = 0
        pend = {}     # q -> [ubg, wt]
        mv_done = set()

        for t in range(n_t):
            c_lo, c_hi = 1024 * t, 1024 * t + 1024
            while next_slab < n_slab and slab_c * next_slab < c_hi + 4 * slab_c:
                emit_slab_load(next_slab)
                next_slab += 1
            if t % (n_t // n_q) == 2:
                q = t // (n_t // n_q)
                if q < n_q:
                    pend[q] = [emit_ubg_load(q), None]

            m1 = m1p.tile([80, 1024], F32, tag="m1")
            # segments: 512-col psum banks, split further at slab boundaries
            for h in range(2):
                c0 = c_lo + 512 * h
                c_end = c0 + 512
                while c0 < c_end:
                    s = c0 // slab_c
                    slot = s % NSLOT
                    c1 = min(c_end, slab_c * (s + 1))
                    nc.tensor.matmul(
                        m1[0:80, c0 - c_lo:c1 - c_lo],
                        w1s2[:, slot, :],
                        ubqt3[:, slot, c0 - slab_c * s:c1 - slab_c * s],
                        start=True, stop=True)
                    c0 = c1

            h1t = h1p.tile([80, 1024], BF16, tag="h1")
            nc.scalar.activation(out=h1t, in_=m1, func=AF.Tanh, scale=0.5)

            m2 = m2p.tile([128, 512], F32, tag="m2")
            nc.tensor.matmul(m2[0:64, :], w2t, h1t[:, 0:512],
                             start=True, stop=True)
            nc.tensor.matmul(m2[64:128, :], w2t, h1t[:, 512:1024],
                             start=True, stop=True)

            h2t = h2p.tile([128, 512], BF16, tag="h2")
            nc.scalar.activation(out=h2t, in_=m2, func=AF.Tanh,
                                 bias=b2c, scale=0.5)

            # mm3: one matmul per h2 tile scores chunk-pair (2t, 2t+1) into
            # psum rows (2j, 2j+1) of group g = t // GP, j = t % GP
            j = t % GP
            if j == 0:
                m3 = m3p.tile([2 * GP, 512], F32, tag="m3")
            nc.tensor.matmul(
                m3, w3v[:, j, :], h2t[0:104, :],
                start=(j == 0), stop=(j == GP - 1))
            if j == GP - 1:
                g = t // GP
                sct = scp.tile([2 * GP, 512], BF16, tag="sct")
                nc.vector.tensor_scalar_add(sct, m3, b3c)
                nc.sync.dma_start(
                    out=bass.AP(tensor=scb_d.tensor,
                                offset=scb_d.offset + g_rows * g,
                                ap=[[512, 2 * GP], [1, 512]]),
                    in_=sct)
                while next_q < n_q and g_rows * (g + 1) >= q_rows * (next_q + 1):
                    pend[next_q][1] = emit_softmax(next_q)
                    next_q += 1

            if t in mv_sched:
                q, u = mv_sched[t]
                emit_mv_quarter(q, u, pend[q][0], pend[q][1])
                mv_done.add((q, u))

        # tail: any quarters not interleaved above
        for q in range(n_q):
            for u in range(4):
                if (q, u) not in mv_done:
                    emit_mv_quarter(q, u, pend[q][0], pend[q][1])

    nc.compile()
    return nc


def host_prep(query_ad, user_behavior, user_behavior_length,
              W1, b1, W2, b2, W3, b3, bc):
    GP = 5
    q = np.asarray(query_ad, dtype=np.float32)
    ub = np.asarray(user_behavior, dtype=np.float32)
    lens = np.asarray(user_behavior_length)
    W1 = np.asarray(W1, dtype=np.float32)
    b1 = np.asarray(b1, dtype=np.float32)
    W2 = np.asarray(W2, dtype=np.float32)
    b2 = np.asarray(b2, dtype=np.float32)
    W3 = np.asarray(W3, dtype=np.float32)
    b3 = np.asarray(b3, dtype=np.float32)
    nb = q.shape[0]
    n_cores = nb // bc

    Wa, Wb, Wc, Wd = W1[0:36], W1[36:72], W1[72:108], W1[108:144]
    # per-slab mm1 lhsT [104, 80]: rows 0-31 = per-batch bias rows R,
    # rows 32-67 = Wb-Wc, rows 68-103 = Wd (matches rhs [S32; ubT; qubT])
    R = q @ (Wa + Wc) + b1[None, :]                        # [nb, 80]
    n_slab = bc // 32
    w1s = np.empty((nb // bc, 104, n_slab, 80), dtype=np.float32)
    for c in range(nb // bc):
        rs = R[bc * c:bc * (c + 1)].reshape(n_slab, 32, 80)
        w1s[c, 0:32] = rs.transpose(1, 0, 2)
        w1s[c, 32:68] = (Wb - Wc)[:, None, :]
        w1s[c, 68:104] = Wd[:, None, :]
    w1s = w1s.reshape(nb // bc, 104, n_slab * 80).astype(NPBF16)
    s32c = np.zeros((32, 32 * T), dtype=np.float32)
    for v in range(32):
        s32c[v, T * v:T * (v + 1)] = 1.0
    s32c = s32c.astype(NPBF16)

    # sigmoid -> tanh folds (see module docstring)
    w2f = np.zeros((80, 64), dtype=np.float32)             # [80,64] zero-padded
    w2f[:, 0:40] = 0.5 * W2
    w2f = w2f.astype(NPBF16)
    b2f = 0.5 * (b2 + 0.5 * W2.sum(axis=0))                # [40]
    w3f = 0.5 * W3[:, 0]                                   # [40]
    b3f = float(b3[0] + 0.5 * W3.sum())

    b2c = np.zeros((128, 1), dtype=np.float32)
    b2c[0:40, 0] = b2f
    b2c[64:104, 0] = b2f
    # mm3 lhsT variants: variant j ([104, 2*GP]) has w3 in column 2j
    # (h2 rows 0-39, even chunk of the pair) and column 2j+1 (rows 64-103)
    w3v = np.zeros((104, GP, 2 * GP), dtype=np.float32)
    for j in range(GP):
        w3v[0:40, j, 2 * j] = w3f
        w3v[64:104, j, 2 * j + 1] = w3f
    w3v = w3v.reshape(104, GP * 2 * GP).astype(NPBF16)
    b3c = np.full((2 * GP, 1), b3f, dtype=np.float32)
    iota = np.arange(T, dtype=np.float32).astype(NPBF16)

    ub16 = ub.astype(NPBF16)                               # [nb, T, 36]

    qub16 = (q[:, None, :] * ub).astype(NPBF16)            # [nb, T, 36]
    in_maps = []
    for c in range(n_cores):
        sl = slice(bc * c, bc * (c + 1))
        ubc = ub16[sl].reshape(bc * T, 36)
        ubqt = np.empty((72, bc * T), dtype=NPBF16)
        ubqt[0:36] = ubc.T
        ubqt[36:72] = qub16[sl].reshape(bc * T, 36).T
        in_maps.append({
            "ubqt": ubqt,
            "ubn": np.ascontiguousarray(ubc),
            "w1s": w1s[c], "s32c": s32c,
            "w2t": w2f, "w3v": w3v, "b2c": b2c, "b3c": b3c,
            "lens2": np.ascontiguousarray(
                lens[sl].astype(np.float32).reshape(bc // 128, 128).T
            ).reshape(bc),
            "iota": iota,
        })
    return in_maps


_NC_CACHE = {}


def get_module(bc):
    if bc not in _NC_CACHE:
        _NC_CACHE[bc] = build_module(bc)
    return _NC_CACHE[bc]


def kernel(query_ad, user_behavior, user_behavior_length,
           W1, b1, W2, b2, W3, b3, trace=False):
    bc = B // N_CORES
    nc = get_module(bc)
    in_maps = host_prep(query_ad, user_behavior, user_behavior_length,
                        W1, b1, W2, b2, W3, b3, bc)
    res = run_bass_kernel_spmd(nc, in_maps, core_ids=list(range(N_CORES)),
                               trace=trace)
    outs = [res.results[c]["out"] for c in range(N_CORES)]
    full = np.concatenate(outs, axis=0).reshape(B, 1, 36)
    if trace:
        kernel.last_result = res
    return full
